# revision 1
# baseline (speedup 1.0000x reference)
"""BaseGCN (4-layer GCN + mean-pool + MLP) on 8 Trainium2 NeuronCores.

Strategy: dst-sharded graph parallel with all-SBUF gathers.
  - z (aggregation inputs) kept TRANSPOSED: z^T chunk tables live in SBUF as
    [128 partitions, NEL] where partitions 32c..32c+w hold features of chunk c
    (4 chunks of the global slot space, each <= 32768 slots for int16).
  - GPSIMD ap_gather pulls per-edge columns (messages) feature-major:
    msgT [128, 2048] per "bank" (32 windows x 64 positions per chunk table).
  - PE transposes 32x128 slabs -> edge-major rows, then K=64 window-pure
    matmuls against streamed norm-folded one-hot tiles accumulate
    agg^T [w, 512] in PSUM (symmetric normalization + self loops folded into
    the one-hot values; mean-pool weights folded in for layer 4).
  - Weights/bias/PReLU applied in transposed space; z^T written per-bank to
    DRAM; AllGather exchanges z^T slices between layers.
  - Layer 4 aggregates h3 @ (W4 lw1 lw2) at width 4 directly into pooled
    logits [4, 64]; AllReduce + constant fold finishes the MLP.

GCNConv(x) = A_hat (x W) + b with A_hat = D^-1/2 A D^-1/2 + D^-1 I;
aggregation commutes with the weight matmul so we aggregate at
width min(d_in, d_out): widths 8, 8, 32, 4.
"""

import os
import numpy as np

# ---------------- problem constants (hardcoded per the contract) ----------
N = 100000
E = 1600000
B = 64
NC = 8
NPC = N // NC          # 12500 dst nodes per core
WIN = 16               # nodes per window (one-hot columns / psum sub-window)
CHUNKS = 4             # src-slot chunks (tables); chunk = src_core // 2
CCAP = 64              # max edges per (window, chunk)  -> one K=64 matmul
WPB = 32               # windows per bank (psum bank = 512 node slots)
POSB = WPB * CCAP      # positions per bank per table (2048)
D_IN = 8
DIMS = [8, 32, 64]             # h widths for layers 1..3
AGG_W = [8, 8, 32, 4]          # aggregation widths per layer
NG = B // WIN                  # graph windows for layer 4 (4)
F32 = np.float32


def _bin_windows(sizes_vec):
    """Bin nodes into windows: <= WIN nodes, per-chunk edge load <= CCAP.
    sizes_vec: [n, CHUNKS] int. Returns (win_of, col_of, n_windows)."""
    n = sizes_vec.shape[0]
    tot = sizes_vec.sum(1)
    order = np.argsort(-tot, kind="stable")
    win_of = np.zeros(n, np.int32)
    col_of = np.zeros(n, np.int32)
    loads = []          # per open bin: [4] loads (python lists)
    cnts = []
    ids = []            # bin ids of active bins
    all_cnt = 0
    for v in order:
        s = sizes_vec[v]
        placed = -1
        for ai, b in enumerate(ids):
            if cnts[ai] < WIN:
                l = loads[ai]
                if (l[0] + s[0] <= CCAP and l[1] + s[1] <= CCAP
                        and l[2] + s[2] <= CCAP and l[3] + s[3] <= CCAP):
                    placed = ai
                    break
        if placed < 0:
            ids.append(all_cnt)
            loads.append([0, 0, 0, 0])
            cnts.append(0)
            all_cnt += 1
            placed = len(ids) - 1
        bid = ids[placed]
        win_of[v] = bid
        col_of[v] = cnts[placed]
        l = loads[placed]
        for c in range(4):
            l[c] += int(s[c])
        cnts[placed] += 1
        if cnts[placed] >= WIN or sum(l) >= 4 * CCAP - 8:
            ids.pop(placed)
            loads.pop(placed)
            cnts.pop(placed)
        elif len(ids) > 48:
            ids.pop(0)
            loads.pop(0)
            cnts.pop(0)
    return win_of, col_of, all_cnt


def _preprocess(x, edge_index, batch):
    src = edge_index[0].astype(np.int64)
    dst = edge_index[1].astype(np.int64)
    batch = batch.astype(np.int64)

    deg = np.bincount(dst, minlength=N).astype(F32) + 1.0
    dinv = (1.0 / np.sqrt(deg)).astype(F32)

    allsrc = np.concatenate([src, np.arange(N, dtype=np.int64)])
    alldst = np.concatenate([dst, np.arange(N, dtype=np.int64)])
    allval = np.concatenate([dinv[src] * dinv[dst], dinv * dinv]).astype(F32)

    cnt = np.maximum(np.bincount(batch, minlength=B).astype(F32), 1.0)

    chunk_of_node = (np.arange(N) // NPC) // 2          # [N] 0..3
    e_chunk = chunk_of_node[allsrc].astype(np.int64)
    e_core = (alldst // NPC).astype(np.int64)

    # ---- per-core window binning (vector caps) ----
    win_of = np.zeros(N, np.int32)
    col_of = np.zeros(N, np.int32)
    nwins = []
    for c in range(NC):
        lo, hi = c * NPC, (c + 1) * NPC
        m = (alldst >= lo) & (alldst < hi)
        sizes = np.zeros((NPC, CHUNKS), np.int64)
        np.add.at(sizes, (alldst[m] - lo, e_chunk[m]), 1)
        w_o, c_o, nw = _bin_windows(sizes)
        win_of[lo:hi] = w_o
        col_of[lo:hi] = c_o
        nwins.append(nw)
    W_CNT = int(np.ceil(max(nwins) / WPB) * WPB)
    BANKS = W_CNT // WPB
    SLOTS = W_CNT * WIN
    GSLOTS = NC * SLOTS
    NEL = GSLOTS // CHUNKS                     # slots per chunk table
    assert NEL <= 32768, NEL

    slot_of = ((np.arange(N) // NPC) * SLOTS + win_of * WIN + col_of).astype(np.int64)
    e_srcslot_loc = (slot_of[allsrc] - e_chunk * NEL).astype(np.int16)
    assert (slot_of[allsrc] - e_chunk * NEL < NEL).all()

    idx123 = np.zeros((NC, BANKS, 128, POSB // 16), np.int16)
    oh123 = np.zeros((NC, BANKS, 128, WPB * WIN * 4), np.float16)

    g_of_e = batch[alldst]
    gwin = g_of_e // WIN
    gcol = g_of_e % WIN
    val4 = allval / cnt[g_of_e]

    # ---- per-core streams ----
    T4 = 0
    percore = []
    for c in range(NC):
        m = e_core == c
        ew = win_of[alldst[m]].astype(np.int64)
        ec = e_chunk[m]
        ecol = col_of[alldst[m]]
        esl = e_srcslot_loc[m]
        ev = allval[m]
        # position within (window, chunk)
        key = ew * CHUNKS + ec
        order = np.argsort(key, kind="stable")
        ks = key[order]
        starts = np.searchsorted(ks, np.arange(W_CNT * CHUNKS))
        pos = np.arange(len(ks)) - starts[ks]
        assert pos.max() < CCAP
        w_s, c_s = ks // CHUNKS, ks % CHUNKS
        p = (w_s % WPB) * CCAP + pos              # position in bank-table stream
        b = w_s // WPB
        lane16 = (p % 16).astype(np.int64)
        col16 = (p // 16).astype(np.int64)
        idx123[c, b, 32 * c_s + lane16, col16] = esl[order]
        idx123[c, b, 32 * c_s + 16 + lane16, col16] = esl[order]
        J = (w_s % WPB) // 2
        oh123[c, b, CCAP * (w_s % 2) + pos,
              (J * CHUNKS + c_s) * 32 + WIN * (w_s % 2) + ecol[order]] = ev[order]

        # layer 4: group by (graph window, chunk)
        g4 = gwin[m].astype(np.int64)
        key4 = g4 * CHUNKS + ec
        order4 = np.argsort(key4, kind="stable")
        k4 = key4[order4]
        starts4 = np.searchsorted(k4, np.arange(NG * CHUNKS))
        pos4 = np.arange(len(k4)) - starts4[k4]
        percore.append((m, order4, k4, pos4))
        cnts4 = np.bincount(k4, minlength=NG * CHUNKS)
        T4 = max(T4, int(np.ceil(cnts4.max() / CCAP)))

    if T4 % 2:
        T4 += 1                                 # run pairs must not straddle graph windows
    NT4POS = NG * T4 * CCAP                     # positions per table
    S4 = int(np.ceil(NT4POS / POSB))
    idx4 = np.zeros((NC, S4, 128, POSB // 16), np.int16)
    oh4 = np.zeros((NC, S4, 128, WPB * WIN * 2), np.float16)
    for c in range(NC):
        m, order4, k4, pos4 = percore[c]
        g_s, c_s = k4 // CHUNKS, k4 % CHUNKS
        p = g_s * (T4 * CCAP) + pos4              # position in table stream
        s = p // POSB
        ps = p % POSB
        esl = e_srcslot_loc[m][order4]
        ev4 = val4[m][order4]
        ecol4 = gcol[m][order4]
        idx4[c, s, 32 * c_s + (ps % 16), ps // 16] = esl
        idx4[c, s, 32 * c_s + 16 + (ps % 16), ps // 16] = esl
        J = (ps // (2 * CCAP))
        oh4[c, s, ps % 128, (J * CHUNKS + c_s) * WIN + ecol4] = ev4

    # x in chunk-table layout [128, NEL]
    xtab = np.zeros((128, NEL), F32)
    xs = np.zeros((GSLOTS, D_IN), F32)
    xs[slot_of] = x
    for c in range(CHUNKS):
        xtab[32 * c:32 * c + D_IN, :] = xs[c * NEL:(c + 1) * NEL].T

    cfg = dict(W_CNT=W_CNT, BANKS=BANKS, SLOTS=SLOTS, GSLOTS=GSLOTS, NEL=NEL,
               T4=T4, S4=S4)
    return cfg, xtab, idx123, oh123, idx4, oh4


def _build_program(cfg):
    import concourse.bacc as bacc
    import concourse.tile as tile
    import concourse.bass as bass
    import concourse.mybir as mybir
    from concourse.masks import make_identity
    from contextlib import ExitStack

    dt = mybir.dt
    BANKS, SLOTS, NEL = cfg["BANKS"], cfg["SLOTS"], cfg["NEL"]
    T4, S4 = cfg["T4"], cfg["S4"]
    IDXW = POSB // 16        # 128
    OHW = WPB * WIN * 4      # 2048 (fp16)
    OHW4 = WPB * WIN * 2     # 1024 (fp16)

    nc = bacc.Bacc("TRN2", target_bir_lowering=False, debug=False, num_devices=NC)

    xtab_d = nc.dram_tensor("xtab", [128, NEL], dt.float32, kind="ExternalInput")
    idx123_d = nc.dram_tensor("idx123", [BANKS, 128, IDXW], dt.int16, kind="ExternalInput")
    oh123_d = nc.dram_tensor("oh123", [BANKS, 128, OHW], dt.float16, kind="ExternalInput")
    idx4_d = nc.dram_tensor("idx4", [S4, 128, IDXW], dt.int16, kind="ExternalInput")
    oh4_d = nc.dram_tensor("oh4", [S4, 128, OHW4], dt.float16, kind="ExternalInput")
    Wd = {}
    for i, (ki, ko) in enumerate([(8, 8), (8, 32), (32, 64), (64, 4)]):
        Wd[i] = nc.dram_tensor(f"W{i+1}", [ki, ko], dt.float32, kind="ExternalInput")
    bd, ad = {}, {}
    for i, d in enumerate(DIMS):
        bd[i] = nc.dram_tensor(f"b{i+1}", [d, 1], dt.float32, kind="ExternalInput")
        ad[i] = nc.dram_tensor(f"a{i+1}", [d, 1], dt.float32, kind="ExternalInput")
    cvec_d = nc.dram_tensor("cvec", [4, 1], dt.float32, kind="ExternalInput")
    out_d = nc.dram_tensor("out", [4, B], dt.float32, kind="ExternalOutput")

    AG = mybir.AluOpType

    with tile.TileContext(nc) as tc, ExitStack() as ctx:
        wpool = ctx.enter_context(tc.tile_pool(name="weights", bufs=1))
        dram = ctx.enter_context(tc.tile_pool(name="dram", bufs=1, space="DRAM"))
        sb = ctx.enter_context(tc.tile_pool(name="sb", bufs=3))
        sbB = ctx.enter_context(tc.tile_pool(name="sbB", bufs=2))
        psA = ctx.enter_context(tc.tile_pool(name="psA", bufs=2, space="PSUM"))
        psB = ctx.enter_context(tc.tile_pool(name="psB", bufs=2, space="PSUM"))
        psC = ctx.enter_context(tc.tile_pool(name="psC", bufs=1, space="PSUM"))
        psT = ctx.enter_context(tc.tile_pool(name="psT", bufs=2, space="PSUM"))
        psP = ctx.enter_context(tc.tile_pool(name="psP", bufs=1, space="PSUM"))

        table = wpool.tile([128, NEL], dt.float32, name="table")
        ident = wpool.tile([128, 128], dt.float32, name="ident")
        make_identity(nc, ident[:])

        Wt, bt, at = {}, {}, {}
        for i, (ki, ko) in enumerate([(8, 8), (8, 32), (32, 64), (64, 4)]):
            Wt[i] = wpool.tile([ki, ko], dt.float32, tag=f"w{i}", name=f"wt{i}")
            nc.sync.dma_start(Wt[i][:], Wd[i][:])
        for i, d in enumerate(DIMS):
            bt[i] = wpool.tile([d, 1], dt.float32, tag=f"b{i}", name=f"bt{i}")
            nc.sync.dma_start(bt[i][:], bd[i][:])
            at[i] = wpool.tile([d, 1], dt.float32, tag=f"a{i}", name=f"at{i}")
            nc.sync.dma_start(at[i][:], ad[i][:])
        cvt = wpool.tile([4, 1], dt.float32, name="cvt")
        nc.sync.dma_start(cvt[:], cvec_d[:])

        zownT = {1: dram.tile([8, SLOTS], dt.float32, name="zo1"),
                 2: dram.tile([32, SLOTS], dt.float32, name="zo2"),
                 3: dram.tile([4, SLOTS], dt.float32, name="zo3")}
        zfullT = {1: dram.tile([NC, 8, SLOTS], dt.float32, name="zf1"),
                  2: dram.tile([NC, 32, SLOTS], dt.float32, name="zf2"),
                  3: dram.tile([NC, 4, SLOTS], dt.float32, name="zf3")}
        pool_in = dram.tile([4, B], dt.float32, name="pin")
        pool_out = dram.tile([4, B], dt.float32, name="pout")

        def agg_phase(table, idx_src, oh_src, nseg, w, body, ohw=None):
            """Gather+transpose+reduce for nseg segments; body(seg, J, c,
            lhsT_ap, oh_tile) emits one K=128 matmul per (slab, chunk)."""
            for s in range(nseg):
                idx_t = sb.tile([128, IDXW], dt.int16, tag="idx", name="idx")
                nc.sync.dma_start(idx_t[:], idx_src[s])
                oh_t = sb.tile([128, ohw or OHW], dt.float16, tag="oh", name="oh")
                nc.sync.dma_start(oh_t[:], oh_src[s])
                msgT = sb.tile([128, POSB], dt.float32, tag="msg", name="msg")
                nc.gpsimd.ap_gather(msgT[:], table[:], idx_t[:],
                                    channels=128, num_elems=NEL, d=1,
                                    num_idxs=POSB)
                for sg in range(4):
                    trp = psT.tile([128, 512], dt.float32, tag="trp", name="trp")
                    for jp in range(4):
                        nc.tensor.transpose(
                            trp[:, jp * 128:jp * 128 + 128],
                            msgT[:, 128 * (sg * 4 + jp):128 * (sg * 4 + jp) + 128],
                            ident[:])
                    slabs = sbB.tile([128, 512], dt.float16, tag="slabs", name="slabs")
                    nc.vector.tensor_copy(slabs[:], trp[:])
                    for jp in range(4):
                        J = sg * 4 + jp
                        for c in range(CHUNKS):
                            body(s, J, c,
                                 slabs[:, jp * 128 + 32 * c:jp * 128 + 32 * c + w],
                                 oh_t)

        def layer(l):  # l = 0, 1, 2
            w = AGG_W[l]
            d = DIMS[l]
            if l == 0:
                nc.scalar.dma_start(table[:], xtab_d[:])
            else:
                for c in range(CHUNKS):
                    nc.scalar.dma_start(table[32 * c:32 * c + w, 0:SLOTS],
                                        zfullT[l][2 * c])
                    nc.scalar.dma_start(table[32 * c:32 * c + w, SLOTS:2 * SLOTS],
                                        zfullT[l][2 * c + 1])

            state = {}

            def body(bank, J, c, lhsT, oh_t):
                if J == 0 and c == 0:
                    state["agg"] = psA.tile([w, 512], dt.float32, tag="agg",
                                            name="agg")
                nc.tensor.matmul(state["agg"][:, 32 * J:32 * J + 32],
                                 lhsT=lhsT,
                                 rhs=oh_t[:, (J * 4 + c) * 32:(J * 4 + c) * 32 + 32],
                                 start=(c == 0), stop=(c == CHUNKS - 1))
                if J == WPB // 2 - 1 and c == CHUNKS - 1:
                    bphase(bank, state["agg"])

            def bphase(bank, agg_ps):
                aggs = sbB.tile([w, 512], dt.float32, tag="aggs", name="aggs")
                nc.vector.tensor_copy(aggs[:], agg_ps[:])
                h_ps = psB.tile([d, 512], dt.float32, tag="h", name="h")
                nc.tensor.matmul(h_ps[:], lhsT=Wt[l][:], rhs=aggs[:],
                                 start=True, stop=True)
                neg = sbB.tile([d, 512], dt.float32, tag="neg", name="neg")
                nc.vector.tensor_scalar(neg[:], h_ps[:], bt[l][:], 0.0, AG.add, AG.min)
                nega = sbB.tile([d, 512], dt.float32, tag="nega", name="nega")
                nc.vector.tensor_scalar(nega[:], neg[:], at[l][:], None, AG.mult)
                pos = sbB.tile([d, 512], dt.float32, tag="pos", name="pos")
                nc.vector.tensor_scalar(pos[:], h_ps[:], bt[l][:], 0.0, AG.add, AG.max)
                hT = sbB.tile([d, 512], dt.float32, tag="hT", name="hT")
                nc.vector.tensor_add(hT[:], pos[:], nega[:])
                if l == 2:
                    z4_ps = psC.tile([4, 512], dt.float32, tag="z4", name="z4")
                    nc.tensor.matmul(z4_ps[:], lhsT=Wt[3][:], rhs=hT[:],
                                     start=True, stop=True)
                    z4s = sbB.tile([4, 512], dt.float32, tag="z4s", name="z4s")
                    nc.vector.tensor_copy(z4s[:], z4_ps[:])
                    nc.scalar.dma_start(zownT[3][:, 512 * bank:512 * bank + 512], z4s[:])
                else:
                    nc.scalar.dma_start(
                        zownT[l + 1][:, 512 * bank:512 * bank + 512], hT[:])

            agg_phase(table, idx123_d, oh123_d, BANKS, w, body)
            zkey = l + 1 if l < 2 else 3
            if os.environ.get("GCN_NO_CC"):
                nc.sync.dma_start(zfullT[zkey][0], zownT[zkey][:])
            else:
                nc.gpsimd.collective_compute(
                    "AllGather", AG.bypass, replica_groups=[list(range(NC))],
                    ins=[zownT[zkey][:].opt()], outs=[zfullT[zkey][:].opt()])

        for l in range(3):
            layer(l)

        # ---- layer 4 ----
        for c in range(CHUNKS):
            nc.scalar.dma_start(table[32 * c:32 * c + 4, 0:SLOTS], zfullT[3][2 * c])
            nc.scalar.dma_start(table[32 * c:32 * c + 4, SLOTS:2 * SLOTS],
                                zfullT[3][2 * c + 1])
        assert T4 % 2 == 0
        pool_ps = psP.tile([4, B], dt.float32, name="pool_ps")

        def body4(s, J, c, lhsT, oh_t):
            R = s * (WPB // 2) + J       # run-pair index for this table
            if 2 * R >= NG * T4:
                return
            g = (2 * R) // T4
            start = (2 * R == g * T4) and c == 0
            stop = (2 * R + 2 == (g + 1) * T4) and c == CHUNKS - 1
            nc.tensor.matmul(pool_ps[:, 16 * g:16 * g + 16],
                             lhsT=lhsT,
                             rhs=oh_t[:, (J * 4 + c) * 16:(J * 4 + c) * 16 + 16],
                             start=start, stop=stop)

        agg_phase(table, idx4_d, oh4_d, S4, 4, body4, ohw=OHW4)

        pooled = sbB.tile([4, B], dt.float32, name="pooled")
        nc.vector.tensor_copy(pooled[:], pool_ps[:])
        nc.sync.dma_start(pool_in[:], pooled[:])
        if os.environ.get("GCN_NO_CC"):
            nc.sync.dma_start(pool_out[:], pool_in[:])
        else:
            nc.gpsimd.collective_compute(
                "AllReduce", AG.add, replica_groups=[list(range(NC))],
                ins=[pool_in[:].opt()], outs=[pool_out[:].opt()])
        res = sbB.tile([4, B], dt.float32, name="res")
        nc.sync.dma_start(res[:], pool_out[:])
        res2 = sbB.tile([4, B], dt.float32, name="res2")
        nc.vector.tensor_scalar(res2[:], res[:], cvt[:], None, AG.add)
        nc.sync.dma_start(out_d[:], res2[:])

    nc.compile()
    return nc


def build(inputs):
    """Host preprocessing + program build. Returns (nc, in_maps)."""
    x = np.asarray(inputs["x"], F32)
    edge_index = np.asarray(inputs["edge_index"])
    batch = np.asarray(inputs["batch"])
    W = [np.asarray(inputs[f"W{i}"], F32) for i in range(1, 5)]
    b = [np.asarray(inputs[f"b{i}"], F32) for i in range(1, 5)]
    a = [np.asarray(inputs[f"a{i}"], F32) for i in range(1, 4)]
    lw1 = np.asarray(inputs["lw1"], F32)
    lb1 = np.asarray(inputs["lb1"], F32)
    lw2 = np.asarray(inputs["lw2"], F32)
    lb2 = np.asarray(inputs["lb2"], F32)

    cfg, xtab, idx123, oh123, idx4, oh4 = _preprocess(x, edge_index, batch)

    W4p = (W[3] @ lw1 @ lw2).astype(F32)                     # [64, 4]
    cv = (b[3] @ lw1 @ lw2 + lb1 @ lw2 + lb2).astype(F32)    # [4]

    nc = _build_program(cfg)

    in_maps = []
    for c in range(NC):
        m = dict(
            xtab=xtab, idx123=idx123[c], oh123=oh123[c],
            idx4=idx4[c], oh4=oh4[c],
            W1=W[0], W2=W[1], W3=W[2], W4=W4p,
            b1=b[0].reshape(-1, 1), b2=b[1].reshape(-1, 1), b3=b[2].reshape(-1, 1),
            a1=np.full((8, 1), a[0][0], F32),
            a2=np.full((32, 1), a[1][0], F32),
            a3=np.full((64, 1), a[2][0], F32),
            cvec=cv.reshape(4, 1),
        )
        in_maps.append(m)
    return nc, in_maps


def kernel(**inputs):
    nc, in_maps = build(inputs)
    from concourse.bass_utils import run_bass_kernel_spmd
    res = run_bass_kernel_spmd(nc, in_maps, list(range(NC)))
    outT = res.results[0]["out"]      # [4, B]
    return np.ascontiguousarray(outT.T.astype(F32))          # [B, 4]



# revision 12
# speedup vs baseline: 1.9536x; 1.9536x over previous
"""BaseGCN (4-layer GCN + mean-pool + MLP) on 8 Trainium2 NeuronCores.

Strategy: dst-sharded graph parallel with all-SBUF gathers.
  - z (aggregation inputs) kept TRANSPOSED: z^T chunk tables live in SBUF as
    [128 partitions, NEL] where partitions 32c..32c+w hold features of chunk c
    (4 chunks of the global slot space, each <= 32768 slots for int16).
  - GPSIMD ap_gather pulls per-edge columns (messages) feature-major:
    msgT [128, 2048] per "bank" (32 windows x 64 positions per chunk table).
  - PE transposes 32x128 slabs -> edge-major rows, then K=64 window-pure
    matmuls against streamed norm-folded one-hot tiles accumulate
    agg^T [w, 512] in PSUM (symmetric normalization + self loops folded into
    the one-hot values; mean-pool weights folded in for layer 4).
  - Weights/bias/PReLU applied in transposed space; z^T written per-bank to
    DRAM; AllGather exchanges z^T slices between layers.
  - Layer 4 aggregates h3 @ (W4 lw1 lw2) at width 4 directly into pooled
    logits [4, 64]; AllReduce + constant fold finishes the MLP.

GCNConv(x) = A_hat (x W) + b with A_hat = D^-1/2 A D^-1/2 + D^-1 I;
aggregation commutes with the weight matmul so we aggregate at
width min(d_in, d_out): widths 8, 8, 32, 4.
"""

import os
import numpy as np

# ---------------- problem constants (hardcoded per the contract) ----------
N = 100000
E = 1600000
B = 64
NC = 8
NPC = N // NC          # 12500 dst nodes per core
WIN = 16               # nodes per window (one-hot columns / psum sub-window)
CHUNKS = 4             # src-slot chunks (tables); chunk = src_core // 2
CCAP = 64              # max edges per (window, chunk)  -> one K=64 matmul
WPB = 32               # windows per bank (psum bank = 512 node slots)
POSB = WPB * CCAP      # positions per bank per table (2048)
D_IN = 8
DIMS = [8, 32, 64]             # h widths for layers 1..3
AGG_W = [8, 8, 32, 4]          # aggregation widths per layer
NG = B // WIN                  # graph windows for layer 4 (4)
F32 = np.float32


def _bin_windows(sizes_vec):
    """Bin nodes into windows: <= WIN nodes, per-chunk edge load <= CCAP.
    sizes_vec: [n, CHUNKS] int. Returns (win_of, col_of, n_windows)."""
    n = sizes_vec.shape[0]
    tot = sizes_vec.sum(1)
    order = np.argsort(-tot, kind="stable")
    win_of = np.zeros(n, np.int32)
    col_of = np.zeros(n, np.int32)
    loads = []          # per open bin: [4] loads (python lists)
    cnts = []
    ids = []            # bin ids of active bins
    all_cnt = 0
    for v in order:
        s = sizes_vec[v]
        placed = -1
        for ai, b in enumerate(ids):
            if cnts[ai] < WIN:
                l = loads[ai]
                if (l[0] + s[0] <= CCAP and l[1] + s[1] <= CCAP
                        and l[2] + s[2] <= CCAP and l[3] + s[3] <= CCAP):
                    placed = ai
                    break
        if placed < 0:
            ids.append(all_cnt)
            loads.append([0, 0, 0, 0])
            cnts.append(0)
            all_cnt += 1
            placed = len(ids) - 1
        bid = ids[placed]
        win_of[v] = bid
        col_of[v] = cnts[placed]
        l = loads[placed]
        for c in range(4):
            l[c] += int(s[c])
        cnts[placed] += 1
        if cnts[placed] >= WIN or sum(l) >= 4 * CCAP - 8:
            ids.pop(placed)
            loads.pop(placed)
            cnts.pop(placed)
        elif len(ids) > 48:
            ids.pop(0)
            loads.pop(0)
            cnts.pop(0)
    return win_of, col_of, all_cnt


def _preprocess(x, edge_index, batch):
    src = edge_index[0].astype(np.int64)
    dst = edge_index[1].astype(np.int64)
    batch = batch.astype(np.int64)

    deg = np.bincount(dst, minlength=N).astype(F32) + 1.0
    dinv = (1.0 / np.sqrt(deg)).astype(F32)

    allsrc = np.concatenate([src, np.arange(N, dtype=np.int64)])
    alldst = np.concatenate([dst, np.arange(N, dtype=np.int64)])
    allval = np.concatenate([dinv[src] * dinv[dst], dinv * dinv]).astype(F32)

    cnt = np.maximum(np.bincount(batch, minlength=B).astype(F32), 1.0)

    chunk_of_node = (np.arange(N) // NPC) // 2          # [N] 0..3
    e_chunk = chunk_of_node[allsrc].astype(np.int64)
    e_core = (alldst // NPC).astype(np.int64)

    # ---- per-core window binning (vector caps) ----
    win_of = np.zeros(N, np.int32)
    col_of = np.zeros(N, np.int32)
    nwins = []
    for c in range(NC):
        lo, hi = c * NPC, (c + 1) * NPC
        m = (alldst >= lo) & (alldst < hi)
        sizes = np.zeros((NPC, CHUNKS), np.int64)
        np.add.at(sizes, (alldst[m] - lo, e_chunk[m]), 1)
        w_o, c_o, nw = _bin_windows(sizes)
        win_of[lo:hi] = w_o
        col_of[lo:hi] = c_o
        nwins.append(nw)
    W_CNT = int(np.ceil(max(nwins) / WPB) * WPB)
    BANKS = W_CNT // WPB
    SLOTS = W_CNT * WIN
    GSLOTS = NC * SLOTS
    NEL = GSLOTS // CHUNKS                     # slots per chunk table
    assert NEL <= 32768, NEL

    slot_of = ((np.arange(N) // NPC) * SLOTS + win_of * WIN + col_of).astype(np.int64)
    e_srcslot_loc = (slot_of[allsrc] - e_chunk * NEL).astype(np.int16)
    assert (slot_of[allsrc] - e_chunk * NEL < NEL).all()

    idx123 = np.zeros((NC, BANKS, 128, POSB // 16), np.int16)
    oh123 = np.zeros((NC, BANKS, 128, WPB * WIN * 4), np.float16)

    # ---- per-core streams ----
    for c in range(NC):
        m = e_core == c
        ew = win_of[alldst[m]].astype(np.int64)
        ec = e_chunk[m]
        ecol = col_of[alldst[m]]
        esl = e_srcslot_loc[m]
        ev = allval[m]
        # position within (window, chunk)
        key = ew * CHUNKS + ec
        order = np.argsort(key, kind="stable")
        ks = key[order]
        starts = np.searchsorted(ks, np.arange(W_CNT * CHUNKS))
        pos = np.arange(len(ks)) - starts[ks]
        assert pos.max() < CCAP
        w_s, c_s = ks // CHUNKS, ks % CHUNKS
        p = (w_s % WPB) * CCAP + pos              # position in bank-table stream
        b = w_s // WPB
        lane16 = (p % 16).astype(np.int64)
        col16 = (p // 16).astype(np.int64)
        idx123[c, b, 32 * c_s + lane16, col16] = esl[order]
        idx123[c, b, 32 * c_s + 16 + lane16, col16] = esl[order]
        J = (w_s % WPB) // 2
        oh123[c, b, CCAP * (w_s % 2) + pos,
              (J * CHUNKS + c_s) * 32 + WIN * (w_s % 2) + ecol[order]] = ev[order]

    # layer-4 pooling one-hot: poolw[slot, g] = 1/cnt[g(node)] in slot space,
    # laid out per (bank, 128-col block): [BANKS, 4, 128, B] fp16
    poolw = np.zeros((NC, BANKS, 128, 4, B), np.float16)
    sl_loc = win_of * WIN + col_of                # slot within core
    for c in range(NC):
        nodes = np.arange(c * NPC, (c + 1) * NPC)
        s = sl_loc[nodes]
        g = batch[nodes]
        poolw[c, s // 512, s % 128, (s % 512) // 128, g] = \
            (1.0 / cnt[g]).astype(np.float16)

    # x in chunk-table layout [128, NEL]
    xtab = np.zeros((128, NEL), F32)
    xs = np.zeros((GSLOTS, D_IN), F32)
    xs[slot_of] = x
    for c in range(CHUNKS):
        xtab[32 * c:32 * c + D_IN, :] = xs[c * NEL:(c + 1) * NEL].T

    cfg = dict(W_CNT=W_CNT, BANKS=BANKS, SLOTS=SLOTS, GSLOTS=GSLOTS, NEL=NEL)
    return cfg, xtab, idx123, oh123, poolw


def _build_program(cfg):
    import concourse.bacc as bacc
    import concourse.tile as tile
    import concourse.bass as bass
    import concourse.mybir as mybir
    from concourse.masks import make_identity
    from contextlib import ExitStack

    dt = mybir.dt
    BANKS, SLOTS, NEL = cfg["BANKS"], cfg["SLOTS"], cfg["NEL"]
    IDXW = POSB // 16        # 128
    OHW = WPB * WIN * 4      # 2048 (fp16)

    nc = bacc.Bacc("TRN2", target_bir_lowering=False, debug=False, num_devices=NC)

    xtab_d = nc.dram_tensor("xtab", [128, NEL], dt.float32, kind="ExternalInput")
    idx123_d = nc.dram_tensor("idx123", [BANKS, 128, IDXW], dt.int16, kind="ExternalInput")
    oh123_d = nc.dram_tensor("oh123", [BANKS, 128, OHW], dt.float16, kind="ExternalInput")
    poolw_d = nc.dram_tensor("poolw", [BANKS, 128, 4, B], dt.float16,
                             kind="ExternalInput")
    Wd = {}
    for i, (ki, ko) in enumerate([(8, 8), (8, 32), (32, 64), (64, 4)]):
        Wd[i] = nc.dram_tensor(f"W{i+1}", [ki, ko], dt.float32, kind="ExternalInput")
    bd, ad = {}, {}
    for i, d in enumerate(DIMS):
        bd[i] = nc.dram_tensor(f"b{i+1}", [d, 1], dt.float32, kind="ExternalInput")
        ad[i] = nc.dram_tensor(f"a{i+1}", [d, 1], dt.float32, kind="ExternalInput")
    cvec_d = nc.dram_tensor("cvec", [4, 1], dt.float32, kind="ExternalInput")
    out_d = nc.dram_tensor("out", [4, B], dt.float32, kind="ExternalOutput")

    AG = mybir.AluOpType

    with tile.TileContext(nc) as tc, ExitStack() as ctx:
        wpool = ctx.enter_context(tc.tile_pool(name="weights", bufs=1))
        dram = ctx.enter_context(tc.tile_pool(name="dram", bufs=1, space="DRAM"))
        sb = ctx.enter_context(tc.tile_pool(name="sb", bufs=3))
        sbB = ctx.enter_context(tc.tile_pool(name="sbB", bufs=2))
        psA = ctx.enter_context(tc.tile_pool(name="psA", bufs=2, space="PSUM"))
        psB = ctx.enter_context(tc.tile_pool(name="psB", bufs=2, space="PSUM"))
        psC = ctx.enter_context(tc.tile_pool(name="psC", bufs=1, space="PSUM"))
        psT = ctx.enter_context(tc.tile_pool(name="psT", bufs=2, space="PSUM"))
        psP = ctx.enter_context(tc.tile_pool(name="psP", bufs=1, space="PSUM"))

        table = wpool.tile([128, NEL], dt.float32, name="table")
        ident = wpool.tile([128, 128], dt.float32, name="ident")
        make_identity(nc, ident[:])

        Wt, bt, at = {}, {}, {}
        for i, (ki, ko) in enumerate([(8, 8), (8, 32), (32, 64), (64, 4)]):
            Wt[i] = wpool.tile([ki, ko], dt.float32, tag=f"w{i}", name=f"wt{i}")
            nc.sync.dma_start(Wt[i][:], Wd[i][:])
        for i, d in enumerate(DIMS):
            bt[i] = wpool.tile([d, 1], dt.float32, tag=f"b{i}", name=f"bt{i}")
            nc.sync.dma_start(bt[i][:], bd[i][:])
            at[i] = wpool.tile([d, 1], dt.float32, tag=f"a{i}", name=f"at{i}")
            nc.sync.dma_start(at[i][:], ad[i][:])
        cvt = wpool.tile([4, 1], dt.float32, name="cvt")
        nc.sync.dma_start(cvt[:], cvec_d[:])

        zownT = {1: dram.tile([8, SLOTS], dt.float32, name="zo1"),
                 2: dram.tile([32, SLOTS], dt.float32, name="zo2"),
                 3: dram.tile([4, SLOTS], dt.float32, name="zo3")}
        zfullT = {1: dram.tile([NC, 8, SLOTS], dt.float32, name="zf1"),
                  2: dram.tile([NC, 32, SLOTS], dt.float32, name="zf2"),
                  3: dram.tile([NC, 4, SLOTS], dt.float32, name="zf3")}
        pool_in = dram.tile([4, B], dt.float32, name="pin")
        pool_out = dram.tile([4, B], dt.float32, name="pout")

        def agg_phase(table, idx_src, oh_src, nseg, w, body, ohw=None):
            """Gather+transpose+reduce for nseg segments; body(seg, J, c,
            lhsT_ap, oh_tile) emits one K=128 matmul per (slab, chunk)."""
            for s in range(nseg):
                idx_t = sb.tile([128, IDXW], dt.int16, tag="idx", name="idx")
                nc.sync.dma_start(idx_t[:], idx_src[s])
                oh_t = sb.tile([128, ohw or OHW], dt.float16, tag="oh", name="oh")
                nc.sync.dma_start(oh_t[:], oh_src[s])
                msgT = sb.tile([128, POSB], dt.float32, tag="msg", name="msg")
                nc.gpsimd.ap_gather(msgT[:], table[:], idx_t[:],
                                    channels=128, num_elems=NEL, d=1,
                                    num_idxs=POSB)
                for sg in range(4):
                    trp = psT.tile([128, 512], dt.float32, tag="trp", name="trp")
                    for jp in range(4):
                        nc.tensor.transpose(
                            trp[:, jp * 128:jp * 128 + 128],
                            msgT[:, 128 * (sg * 4 + jp):128 * (sg * 4 + jp) + 128],
                            ident[:])
                    slabs = sbB.tile([128, 512], dt.float16, tag="slabs", name="slabs")
                    nc.vector.tensor_copy(slabs[:], trp[:])
                    for jp in range(4):
                        J = sg * 4 + jp
                        for c in range(CHUNKS):
                            body(s, J, c,
                                 slabs[:, jp * 128 + 32 * c:jp * 128 + 32 * c + w],
                                 oh_t)

        def layer(l):  # l = 0, 1, 2
            w = AGG_W[l]
            d = DIMS[l]
            if l == 0:
                nc.scalar.dma_start(table[:], xtab_d[:])
            else:
                for c in range(CHUNKS):
                    nc.scalar.dma_start(table[32 * c:32 * c + w, 0:SLOTS],
                                        zfullT[l][2 * c])
                    nc.scalar.dma_start(table[32 * c:32 * c + w, SLOTS:2 * SLOTS],
                                        zfullT[l][2 * c + 1])

            state = {}

            def body(bank, J, c, lhsT, oh_t):
                if J == 0 and c == 0:
                    state["agg"] = psA.tile([w, 512], dt.float32, tag="agg",
                                            name="agg")
                nc.tensor.matmul(state["agg"][:, 32 * J:32 * J + 32],
                                 lhsT=lhsT,
                                 rhs=oh_t[:, (J * 4 + c) * 32:(J * 4 + c) * 32 + 32],
                                 start=(c == 0), stop=(c == CHUNKS - 1))
                if J == WPB // 2 - 1 and c == CHUNKS - 1:
                    bphase(bank, state["agg"])

            def bphase(bank, agg_ps):
                aggs = sbB.tile([w, 512], dt.float32, tag="aggs", name="aggs")
                nc.vector.tensor_copy(aggs[:], agg_ps[:])
                h_ps = psB.tile([d, 512], dt.float32, tag="h", name="h")
                nc.tensor.matmul(h_ps[:], lhsT=Wt[l][:], rhs=aggs[:],
                                 start=True, stop=True)
                neg = sbB.tile([d, 512], dt.float32, tag="neg", name="neg")
                nc.vector.tensor_scalar(neg[:], h_ps[:], bt[l][:], 0.0, AG.add, AG.min)
                nega = sbB.tile([d, 512], dt.float32, tag="nega", name="nega")
                nc.vector.tensor_scalar(nega[:], neg[:], at[l][:], None, AG.mult)
                pos = sbB.tile([d, 512], dt.float32, tag="pos", name="pos")
                nc.vector.tensor_scalar(pos[:], h_ps[:], bt[l][:], 0.0, AG.add, AG.max)
                hT = sbB.tile([d, 512], dt.float32, tag="hT", name="hT")
                nc.vector.tensor_add(hT[:], pos[:], nega[:])
                if l == 2:
                    z4_ps = psC.tile([4, 512], dt.float32, tag="z4", name="z4")
                    nc.tensor.matmul(z4_ps[:], lhsT=Wt[3][:], rhs=hT[:],
                                     start=True, stop=True)
                    z4s = sbB.tile([4, 512], dt.float32, tag="z4s", name="z4s")
                    nc.vector.tensor_copy(z4s[:], z4_ps[:])
                    nc.scalar.dma_start(zownT[3][:, 512 * bank:512 * bank + 512], z4s[:])
                else:
                    nc.scalar.dma_start(
                        zownT[l + 1][:, 512 * bank:512 * bank + 512], hT[:])

            agg_phase(table, idx123_d, oh123_d, BANKS, w, body)
            zkey = l + 1 if l < 2 else 3
            if os.environ.get("GCN_NO_CC"):
                nc.sync.dma_start(zfullT[zkey][0], zownT[zkey][:])
            else:
                nc.gpsimd.collective_compute(
                    "AllGather", AG.bypass, replica_groups=[list(range(NC))],
                    ins=[zownT[zkey][:].opt()], outs=[zfullT[zkey][:].opt()])

        for l in range(3):
            layer(l)

        # ---- layer 4: per-node agg of z4 (shares idx123/oh123), then pool ----
        for c in range(CHUNKS):
            nc.scalar.dma_start(table[32 * c:32 * c + 4, 0:SLOTS], zfullT[3][2 * c])
            nc.scalar.dma_start(table[32 * c:32 * c + 4, SLOTS:2 * SLOTS],
                                zfullT[3][2 * c + 1])
        pool_ps = psP.tile([4, B], dt.float32, name="pool_ps")
        state4 = {}

        def body4(bank, J, c, lhsT, oh_t):
            if J == 0 and c == 0:
                state4["agg"] = psA.tile([4, 512], dt.float32, tag="agg",
                                         name="agg4")
            nc.tensor.matmul(state4["agg"][:, 32 * J:32 * J + 32],
                             lhsT=lhsT,
                             rhs=oh_t[:, (J * 4 + c) * 32:(J * 4 + c) * 32 + 32],
                             start=(c == 0), stop=(c == CHUNKS - 1))
            if J == WPB // 2 - 1 and c == CHUNKS - 1:
                pphase(bank, state4["agg"])

        def pphase(bank, agg_ps):
            aggs = sbB.tile([4, 512], dt.float32, tag="aggs", name="agg4s")
            nc.vector.tensor_copy(aggs[:], agg_ps[:])
            pw = sb.tile([128, 4, B], dt.float16, tag="pw", name="pw")
            nc.sync.dma_start(pw[:], poolw_d[bank])
            for j in range(4):
                trp = psT.tile([128, 4], dt.float32, tag="trp", name="trp4")
                nc.tensor.transpose(trp[:], aggs[:, 128 * j:128 * j + 128],
                                    ident[:4, :4])
                trs = sbB.tile([128, 4], dt.float16, tag="trs", name="trs")
                nc.vector.tensor_copy(trs[:], trp[:])
                nc.tensor.matmul(pool_ps[:],
                                 lhsT=trs[:], rhs=pw[:, j, :],
                                 start=(bank == 0 and j == 0),
                                 stop=(bank == BANKS - 1 and j == 3))

        agg_phase(table, idx123_d, oh123_d, BANKS, 4, body4)

        pooled = sbB.tile([4, B], dt.float32, name="pooled")
        nc.vector.tensor_copy(pooled[:], pool_ps[:])
        nc.sync.dma_start(pool_in[:], pooled[:])
        if os.environ.get("GCN_NO_CC"):
            nc.sync.dma_start(pool_out[:], pool_in[:])
        else:
            nc.gpsimd.collective_compute(
                "AllReduce", AG.add, replica_groups=[list(range(NC))],
                ins=[pool_in[:].opt()], outs=[pool_out[:].opt()])
        res = sbB.tile([4, B], dt.float32, name="res")
        nc.sync.dma_start(res[:], pool_out[:])
        res2 = sbB.tile([4, B], dt.float32, name="res2")
        nc.vector.tensor_scalar(res2[:], res[:], cvt[:], None, AG.add)
        nc.sync.dma_start(out_d[:], res2[:])

    nc.compile()
    return nc


def build(inputs):
    """Host preprocessing + program build. Returns (nc, in_maps)."""
    x = np.asarray(inputs["x"], F32)
    edge_index = np.asarray(inputs["edge_index"])
    batch = np.asarray(inputs["batch"])
    W = [np.asarray(inputs[f"W{i}"], F32) for i in range(1, 5)]
    b = [np.asarray(inputs[f"b{i}"], F32) for i in range(1, 5)]
    a = [np.asarray(inputs[f"a{i}"], F32) for i in range(1, 4)]
    lw1 = np.asarray(inputs["lw1"], F32)
    lb1 = np.asarray(inputs["lb1"], F32)
    lw2 = np.asarray(inputs["lw2"], F32)
    lb2 = np.asarray(inputs["lb2"], F32)

    cfg, xtab, idx123, oh123, poolw = _preprocess(x, edge_index, batch)

    W4p = (W[3] @ lw1 @ lw2).astype(F32)                     # [64, 4]
    cv = (b[3] @ lw1 @ lw2 + lb1 @ lw2 + lb2).astype(F32)    # [4]

    nc = _build_program(cfg)

    in_maps = []
    for c in range(NC):
        m = dict(
            xtab=xtab, idx123=idx123[c], oh123=oh123[c],
            poolw=poolw[c],
            W1=W[0], W2=W[1], W3=W[2], W4=W4p,
            b1=b[0].reshape(-1, 1), b2=b[1].reshape(-1, 1), b3=b[2].reshape(-1, 1),
            a1=np.full((8, 1), a[0][0], F32),
            a2=np.full((32, 1), a[1][0], F32),
            a3=np.full((64, 1), a[2][0], F32),
            cvec=cv.reshape(4, 1),
        )
        in_maps.append(m)
    return nc, in_maps


def kernel(**inputs):
    nc, in_maps = build(inputs)
    from concourse.bass_utils import run_bass_kernel_spmd
    res = run_bass_kernel_spmd(nc, in_maps, list(range(NC)))
    outT = res.results[0]["out"]      # [4, B]
    return np.ascontiguousarray(outT.T.astype(F32))          # [B, 4]



# revision 42
# speedup vs baseline: 3.3604x; 1.7201x over previous
"""BaseGCN (4-layer GCN + mean-pool + MLP) on 8 Trainium2 NeuronCores.

Strategy: dst-sharded graph parallel, GPSIMD ap_gather message gathering,
PE one-hot scatter matmuls.

  - z tables live in SBUF feature-major: [128 partitions, NEL] where a
    partition-row group holds one src chunk's features.  Layers 1/2/4
    aggregate at width <= 8 and use EIGHT chunks (chunk = src core, 16
    partition rows each) so each gather column serves 8 edges; layer 3
    (width 32) uses FOUR chunks (32 rows each, 2 cores per chunk).
  - dst nodes are packed into "bins" (<= 64 nodes for the 8-chunk group,
    <= 32 for the 4-chunk group) such that every (bin, chunk) has <= 128
    edges; a bin owns one 128-column block of the gather stream, shared
    by all chunks.
  - GPSIMD ap_gather pulls per-edge columns feature-major; PE transposes
    128-col slabs to edge-major; K=128 matmuls against streamed one-hot
    tiles accumulate agg^T [w, 512] per bank in PSUM.
  - Weights/bias/PReLU applied in transposed space; z^T written per-bank
    to DRAM; AllGather exchanges z^T between layers.
  - Layer 4 aggregates z4 = h3 @ (W4 lw1 lw2) at width 4 reusing the
    8-chunk streams, then pools via per-bank transposes + matmuls against
    a static [slot, graph] 1/cnt matrix; AllReduce + constant finishes.

GCNConv(x) = A_hat (x W) + b with A_hat = D^-1/2 A D^-1/2 + D^-1 I;
aggregation commutes with the weight matmul so we aggregate at
width min(d_in, d_out): widths 8, 8, 32, 4.
"""

import os
import numpy as np

# ---------------- problem constants (hardcoded per the contract) ----------
N = 100000
E = 1600000
B = 64
NC = 8
NPC = N // NC          # 12500 dst nodes per core
CAP = 128              # edges per (bin, chunk) == columns per bin
BIN8 = 64              # node columns per bin, 8-chunk group
BIN4 = 32              # node columns per bin, 4-chunk group
NI8 = 1024             # gather columns per bank, 8-chunk group (8 bins)
NI4 = 2048             # gather columns per bank, 4-chunk group (16 bins)
D_IN = 8
DIMS = [8, 32, 64]             # h widths for layers 1..3
AGG_W = [8, 8, 32, 4]          # aggregation widths per layer
F32 = np.float32


def _bin_nodes(sizes, maxn, group):
    """Balanced multiway packing: fix a target bin count, then place each
    node (desc by load) into the feasible bin minimizing the resulting max
    chunk load.  sizes: [n, CH] int.  Returns (bin_of, col_of, n_bins)."""
    n, ch = sizes.shape
    tot = sizes.sum(1)
    target = max(int(np.ceil(n / maxn)),
                 int(np.ceil(sizes.sum(0).max() * 1.035 / CAP)),
                 int(np.ceil(sizes.sum() * 1.03 / (ch * CAP))))
    target = int(np.ceil(target / group) * group)
    order = np.argsort(-tot, kind="stable")
    cap_bins = target + 4 * group
    loads = np.zeros((cap_bins, ch), np.int64)
    cnts = np.zeros(cap_bins, np.int64)
    nbins = target
    bin_of = np.zeros(n, np.int32)
    col_of = np.zeros(n, np.int32)
    for v in order:
        s = sizes[v]
        cand = loads[:nbins] + s
        mx = cand.max(1)
        feas = (cand <= CAP).all(1) & (cnts[:nbins] < maxn)
        if feas.any():
            mx[~feas] = 1 << 30
            bi = int(np.argmin(mx))
        else:
            bi = nbins
            nbins += 1
        bin_of[v] = bi
        col_of[v] = cnts[bi]
        loads[bi] += s
        cnts[bi] += 1
    return bin_of, col_of, nbins


def _preprocess(x, edge_index, batch):
    src = edge_index[0].astype(np.int64)
    dst = edge_index[1].astype(np.int64)
    batch = batch.astype(np.int64)

    deg = np.bincount(dst, minlength=N).astype(F32) + 1.0
    dinv = (1.0 / np.sqrt(deg)).astype(F32)
    dinv2 = (dinv * dinv).astype(F32)

    # edge-only streams for the 8-chunk group (self loops handled via
    # algebraic diagonal terms); self loops stay in-stream for layer 3.
    allsrc = np.concatenate([src, np.arange(N, dtype=np.int64)])
    alldst = np.concatenate([dst, np.arange(N, dtype=np.int64)])
    allval = np.concatenate([dinv[src] * dinv[dst], dinv2]).astype(F32)
    is_edge = np.concatenate([np.ones(len(src), bool), np.zeros(N, bool)])

    cnt = np.maximum(np.bincount(batch, minlength=B).astype(F32), 1.0)

    sc8 = (allsrc // NPC).astype(np.int64)            # src core = 8-chunk id
    sc4 = sc8 // 2                                    # 4-chunk id
    e_core = (alldst // NPC).astype(np.int64)

    # ---- per-core binning ----
    bin8_of = np.zeros(N, np.int32); col8_of = np.zeros(N, np.int32)
    bin4_of = np.zeros(N, np.int32); col4_of = np.zeros(N, np.int32)
    nb8 = []; nb4 = []
    for c in range(NC):
        lo, hi = c * NPC, (c + 1) * NPC
        m = (alldst >= lo) & (alldst < hi)
        me = m & is_edge
        d8 = np.zeros((NPC, 8), np.int64)
        np.add.at(d8, (alldst[me] - lo, sc8[me]), 1)
        b_o, c_o, nb = _bin_nodes(d8, BIN8, 8)
        bin8_of[lo:hi] = b_o; col8_of[lo:hi] = c_o; nb8.append(nb)
        d4 = d8.reshape(NPC, 4, 2).sum(2)
        d4[:, c // 2] += 1                                # self loops
        b_o, c_o, nb = _bin_nodes(d4, BIN4, 16)
        bin4_of[lo:hi] = b_o; col4_of[lo:hi] = c_o; nb4.append(nb)
    NQB = int(np.ceil(max(nb8) / 8) * 8)         # bins56 count (x8 per bank)
    NPB = int(np.ceil(max(nb4) / 16) * 16)       # bins28 count (x16 per bank)
    B8 = NQB // 8
    B4 = NPB // 16
    QS = NQB * BIN8                              # qs slots per core
    PS = NPB * BIN4                              # ps slots per core
    assert QS <= 16384 and PS <= 32768, (QS, PS)

    qs_slot = (bin8_of.astype(np.int64) * BIN8 + col8_of)   # core-local
    ps_slot = (bin4_of.astype(np.int64) * BIN4 + col4_of)

    OHW8 = 8 * 8 * BIN8      # 4096
    OHW4 = 16 * 4 * BIN4     # 2048
    idx8 = np.zeros((NC, B8, 128, NI8 // 16), np.int16)
    idx8p = np.zeros((NC, B8, 128, NI8 // 16), np.int16)
    oh8 = np.zeros((NC, B8, 128, OHW8), np.float16)
    idx4 = np.zeros((NC, B4, 128, NI4 // 16), np.int16)
    oh4 = np.zeros((NC, B4, 128, OHW4), np.float16)

    for c in range(NC):
        m = (e_core == c) & is_edge
        ev = allval[m]
        esrc = allsrc[m]
        edst = alldst[m]
        esc8 = sc8[m]

        # ---- 8-chunk stream (layers 1, 2, 4; edges only) ----
        key = bin8_of[edst].astype(np.int64) * 8 + esc8
        order = np.argsort(key, kind="stable")
        ks = key[order]
        starts = np.searchsorted(ks, np.arange(NQB * 8))
        pos = np.arange(len(ks)) - starts[ks]
        assert pos.max() < CAP
        bn, ch = ks // 8, ks % 8
        col = bn * CAP + pos
        bank, cb = col // NI8, col % NI8
        idx8[c, bank, 16 * ch + cb % 16, cb // 16] = qs_slot[esrc[order]]
        idx8p[c, bank, 16 * ch + cb % 16, cb // 16] = ps_slot[esrc[order]]
        blk = (cb // 128)
        oh8[c, bank, cb % 128,
            (blk * 8 + ch) * BIN8 + col8_of[edst[order]]] = ev[order]

        # ---- 4-chunk stream (layer 3; includes self loops) ----
        m = e_core == c
        ev = allval[m]
        esrc = allsrc[m]
        edst = alldst[m]
        esc4 = sc4[m]
        key = bin4_of[edst].astype(np.int64) * 4 + esc4
        order = np.argsort(key, kind="stable")
        ks = key[order]
        starts = np.searchsorted(ks, np.arange(NPB * 4))
        pos = np.arange(len(ks)) - starts[ks]
        assert pos.max() < CAP
        bn, ch = ks // 4, ks % 4
        col = bn * CAP + pos
        bank, cb = col // NI4, col % NI4
        loc = (esrc[order] // NPC) % 2 * QS + qs_slot[esrc[order]]
        idx4[c, bank, 32 * ch + cb % 16, cb // 16] = loc
        idx4[c, bank, 32 * ch + 16 + cb % 16, cb // 16] = loc
        blk = (cb // 128)
        oh4[c, bank, cb % 128,
            (blk * 4 + ch) * BIN4 + col4_of[edst[order]]] = ev[order]

    # layer-4 pooling one-hot in QS space: [B8 banks, 128, 4, B] fp16
    poolw = np.zeros((NC, B8, 128, 4, B), np.float16)
    # layer-4 self-loop pool (applied to z4 in PS space during layer 3):
    # value dinv2/cnt at [B4 banks, 128, 4, B]
    pool2w = np.zeros((NC, B4, 128, 4, B), np.float16)
    # xself: x * dinv2 in QS columns (layer-1 diagonal term)
    xself = np.zeros((NC, 8, QS), F32)
    # dinv2 in QS columns, replicated to 8 rows (layer-2 diagonal term)
    d2q = np.zeros((NC, 8, QS), F32)
    # x in 8-chunk table layout [128, QS]
    xtab = np.zeros((128, QS), F32)
    for c in range(NC):
        nodes = np.arange(c * NPC, (c + 1) * NPC)
        s = qs_slot[nodes]
        g = batch[nodes]
        poolw[c, s // 512, s % 128, (s % 512) // 128, g] = \
            (1.0 / cnt[g]).astype(np.float16)
        sp = ps_slot[nodes]
        pool2w[c, sp // 512, sp % 128, (sp % 512) // 128, g] = \
            (dinv2[nodes] / cnt[g]).astype(np.float16)
        xself[c, :, s] = x[nodes] * dinv2[nodes][:, None]
        d2q[c, :, s] = np.repeat(dinv2[nodes][:, None], 8, 1)
        xtab[16 * c:16 * c + D_IN, s] = x[nodes].T

    cfg = dict(B8=B8, B4=B4, QS=QS, PS=PS)
    return (cfg, xtab, idx8, idx8p, oh8, idx4, oh4, poolw, pool2w,
            xself, d2q)


def _build_program(cfg):
    import concourse.bacc as bacc
    import concourse.tile as tile
    import concourse.bass as bass
    import concourse.mybir as mybir
    from concourse.masks import make_identity
    from contextlib import ExitStack

    dt = mybir.dt
    B8, B4, QS, PS = cfg["B8"], cfg["B4"], cfg["QS"], cfg["PS"]
    NEL4 = 2 * QS
    OHW8 = 8 * 8 * BIN8
    OHW4 = 16 * 4 * BIN4

    nc = bacc.Bacc("TRN2", target_bir_lowering=False, debug=False, num_devices=NC)

    xtab_d = nc.dram_tensor("xtab", [128, QS], dt.float32, kind="ExternalInput")
    idx8_d = nc.dram_tensor("idx8", [B8, 128, NI8 // 16], dt.int16, kind="ExternalInput")
    idx8p_d = nc.dram_tensor("idx8p", [B8, 128, NI8 // 16], dt.int16, kind="ExternalInput")
    oh8_d = nc.dram_tensor("oh8", [B8, 128, OHW8], dt.float16, kind="ExternalInput")
    idx4_d = nc.dram_tensor("idx4", [B4, 128, NI4 // 16], dt.int16, kind="ExternalInput")
    oh4_d = nc.dram_tensor("oh4", [B4, 128, OHW4], dt.float16, kind="ExternalInput")
    poolw_d = nc.dram_tensor("poolw", [B8, 128, 4, B], dt.float16,
                             kind="ExternalInput")
    pool2w_d = nc.dram_tensor("pool2w", [B4, 128, 4, B], dt.float16,
                              kind="ExternalInput")
    xself_d = nc.dram_tensor("xself", [8, QS], dt.float32, kind="ExternalInput")
    d2q_d = nc.dram_tensor("d2q", [8, QS], dt.float32, kind="ExternalInput")
    Wd = {}
    for i, (ki, ko) in enumerate([(8, 8), (8, 32), (32, 64), (64, 4)]):
        Wd[i] = nc.dram_tensor(f"W{i+1}", [ki, ko], dt.float32, kind="ExternalInput")
    bd, ad = {}, {}
    for i, d in enumerate(DIMS):
        bd[i] = nc.dram_tensor(f"b{i+1}", [d, 1], dt.float32, kind="ExternalInput")
        ad[i] = nc.dram_tensor(f"a{i+1}", [d, 1], dt.float32, kind="ExternalInput")
    cvec_d = nc.dram_tensor("cvec", [4, 1], dt.float32, kind="ExternalInput")
    out_d = nc.dram_tensor("out", [4, B], dt.float32, kind="ExternalOutput")

    AG = mybir.AluOpType

    with tile.TileContext(nc) as tc, ExitStack() as ctx:
        wpool = ctx.enter_context(tc.tile_pool(name="weights", bufs=1))
        dram = ctx.enter_context(tc.tile_pool(name="dram", bufs=1, space="DRAM"))
        sb = ctx.enter_context(tc.tile_pool(name="sb", bufs=3))
        sbB = ctx.enter_context(tc.tile_pool(name="sbB", bufs=2))
        psA = ctx.enter_context(tc.tile_pool(name="psA", bufs=2, space="PSUM"))
        psB = ctx.enter_context(tc.tile_pool(name="psB", bufs=1, space="PSUM"))
        psC = ctx.enter_context(tc.tile_pool(name="psC", bufs=1, space="PSUM"))
        psT = ctx.enter_context(tc.tile_pool(name="psT", bufs=2, space="PSUM"))
        psT2 = ctx.enter_context(tc.tile_pool(name="psT2", bufs=1, space="PSUM"))
        psP = ctx.enter_context(tc.tile_pool(name="psP", bufs=1, space="PSUM"))

        table = wpool.tile([128, NEL4], dt.float32, name="table")
        ident = wpool.tile([128, 128], dt.float32, name="ident")
        make_identity(nc, ident[:])

        Wt, bt, at = {}, {}, {}
        for i, (ki, ko) in enumerate([(8, 8), (8, 32), (32, 64), (64, 4)]):
            Wt[i] = wpool.tile([ki, ko], dt.float32, tag=f"w{i}", name=f"wt{i}")
            nc.sync.dma_start(Wt[i][:], Wd[i][:])
        for i, d in enumerate(DIMS):
            bt[i] = wpool.tile([d, 1], dt.float32, tag=f"b{i}", name=f"bt{i}")
            nc.sync.dma_start(bt[i][:], bd[i][:])
            at[i] = wpool.tile([d, 1], dt.float32, tag=f"a{i}", name=f"at{i}")
            nc.sync.dma_start(at[i][:], ad[i][:])
        cvt = wpool.tile([4, 1], dt.float32, name="cvt")
        nc.sync.dma_start(cvt[:], cvec_d[:])

        zownT = {1: dram.tile([8, QS], dt.float32, name="zo1"),
                 2: dram.tile([32, QS], dt.float32, name="zo2"),
                 3: dram.tile([4, PS], dt.float32, name="zo3")}
        zfullT = {1: dram.tile([NC, 8, QS], dt.float32, name="zf1"),
                  2: dram.tile([NC, 32, QS], dt.float32, name="zf2"),
                  3: dram.tile([NC, 4, PS], dt.float32, name="zf3")}
        pool_in = dram.tile([4, B], dt.float32, name="pin")
        pool_out = dram.tile([4, B], dt.float32, name="pout")

        def agg_phase(nel, idx_src, oh_src, nbanks, ni, chn, w, bincols, ohw,
                      body):
            """Gather+transpose+scatter for nbanks banks.  body(bank, t, c,
            lhsT_ap, oh_tile) emits one K=128 matmul per (block, chunk)."""
            rw = 128 // chn
            idxw = ni // 16
            nblk = ni // 128
            for s in range(nbanks):
                idx_t = sb.tile([128, idxw], dt.int16, tag="idx", name="idx")
                nc.sync.dma_start(idx_t[:], idx_src[s])
                oh_t = sb.tile([128, ohw], dt.float16, tag="oh", name="oh")
                nc.sync.dma_start(oh_t[:], oh_src[s])
                msgT = sb.tile([128, ni], dt.float32, tag="msg", name="msg")
                nc.gpsimd.ap_gather(msgT[:], table[:, 0:nel], idx_t[:],
                                    channels=128, num_elems=nel, d=1,
                                    num_idxs=ni)
                for sg in range(nblk // 4):
                    trp = psT.tile([128, 512], dt.float32, tag="trp", name="trp")
                    for jp in range(4):
                        nc.tensor.transpose(
                            trp[:, jp * 128:jp * 128 + 128],
                            msgT[:, 128 * (sg * 4 + jp):128 * (sg * 4 + jp) + 128],
                            ident[:])
                    slabs = sbB.tile([128, 512], dt.float16, tag="slabs",
                                     name="slabs")
                    nc.vector.tensor_copy(slabs[:], trp[:])
                    for jp in range(4):
                        t = sg * 4 + jp
                        for c in range(chn):
                            body(s, t, c,
                                 slabs[:, jp * 128 + rw * c:jp * 128 + rw * c + w],
                                 oh_t)

        def layer(l):  # l = 0, 1, 2
            w = AGG_W[l]
            d = DIMS[l]
            if l == 0:
                nc.scalar.dma_start(table[:, 0:QS], xtab_d[:])
            elif l == 1:
                for c in range(NC):
                    nc.scalar.dma_start(table[16 * c:16 * c + 8, 0:QS],
                                        zfullT[1][c])
            else:
                for c in range(4):
                    nc.scalar.dma_start(table[32 * c:32 * c + 32, 0:QS],
                                        zfullT[2][2 * c])
                    nc.scalar.dma_start(table[32 * c:32 * c + 32, QS:2 * QS],
                                        zfullT[2][2 * c + 1])

            chn = 4 if l == 2 else 8
            bincols = BIN4 if l == 2 else BIN8
            nblk = (NI4 if l == 2 else NI8) // 128
            state = {}

            def body(bank, t, c, lhsT, oh_t):
                if t == 0 and c == 0:
                    state["agg"] = psA.tile([w, 512], dt.float32, tag="agg",
                                            name="agg")
                nc.tensor.matmul(
                    state["agg"][:, bincols * t:bincols * t + bincols],
                    lhsT=lhsT,
                    rhs=oh_t[:, (t * chn + c) * bincols:(t * chn + c + 1) * bincols],
                    start=(c == 0), stop=(c == chn - 1))
                if t == nblk - 1 and c == chn - 1:
                    bphase(bank, state["agg"])

            def bphase(bank, agg_ps):
                aggs = sbB.tile([w, 512], dt.float32, tag="aggs", name="aggs")
                if l == 0:
                    xs = sb.tile([8, 512], dt.float32, tag="xs", name="xs")
                    nc.sync.dma_start(xs[:], xself_d[:, 512 * bank:512 * bank + 512])
                    nc.vector.tensor_add(aggs[:], agg_ps[:], xs[:])
                elif l == 1:
                    zs = sb.tile([8, 512], dt.float32, tag="xs", name="zs")
                    nc.sync.dma_start(zs[:], zownT[1][:, 512 * bank:512 * bank + 512])
                    d2 = sb.tile([8, 512], dt.float32, tag="d2", name="d2")
                    nc.sync.dma_start(d2[:], d2q_d[:, 512 * bank:512 * bank + 512])
                    zsd = sbB.tile([8, 512], dt.float32, tag="zsd", name="zsd")
                    nc.vector.tensor_mul(zsd[:], zs[:], d2[:])
                    nc.vector.tensor_add(aggs[:], agg_ps[:], zsd[:])
                else:
                    nc.vector.tensor_copy(aggs[:], agg_ps[:])
                h_ps = psB.tile([d, 512], dt.float32, tag="h", name="h")
                nc.tensor.matmul(h_ps[:], lhsT=Wt[l][:], rhs=aggs[:],
                                 start=True, stop=True)
                neg = sbB.tile([d, 512], dt.float32, tag="neg", name="neg")
                nc.vector.tensor_scalar(neg[:], h_ps[:], bt[l][:], 0.0, AG.add, AG.min)
                nega = sbB.tile([d, 512], dt.float32, tag="nega", name="nega")
                nc.vector.tensor_scalar(nega[:], neg[:], at[l][:], None, AG.mult)
                pos = sbB.tile([d, 512], dt.float32, tag="pos", name="pos")
                nc.vector.tensor_scalar(pos[:], h_ps[:], bt[l][:], 0.0, AG.add, AG.max)
                hT = sbB.tile([d, 512], dt.float32, tag="hT", name="hT")
                nc.vector.tensor_add(hT[:], pos[:], nega[:])
                if l == 2:
                    z4_ps = psC.tile([4, 512], dt.float32, tag="z4", name="z4")
                    nc.tensor.matmul(z4_ps[:], lhsT=Wt[3][:], rhs=hT[:],
                                     start=True, stop=True)
                    z4s = sbB.tile([4, 512], dt.float32, tag="z4s", name="z4s")
                    nc.vector.tensor_copy(z4s[:], z4_ps[:])
                    nc.scalar.dma_start(zownT[3][:, 512 * bank:512 * bank + 512],
                                        z4s[:])
                    # layer-4 self-loop term: pool dinv2*z4 directly
                    p2 = sb.tile([128, 4, B], dt.float16, tag="pw", name="p2w")
                    nc.sync.dma_start(p2[:], pool2w_d[bank])
                    for j in range(4):
                        trp2 = psT2.tile([128, 4], dt.float32, tag="trp2",
                                         name="trp2")
                        nc.tensor.transpose(trp2[:],
                                            z4s[:, 128 * j:128 * j + 128],
                                            ident[:4, :4])
                        trs2 = sbB.tile([128, 4], dt.float16, tag="trs",
                                        name="trs2")
                        nc.vector.tensor_copy(trs2[:], trp2[:])
                        nc.tensor.matmul(pool_ps[:],
                                         lhsT=trs2[:], rhs=p2[:, j, :],
                                         start=(bank == 0 and j == 0),
                                         stop=False)
                else:
                    nc.scalar.dma_start(
                        zownT[l + 1][:, 512 * bank:512 * bank + 512], hT[:])

            if l == 2:
                agg_phase(NEL4, idx4_d, oh4_d, B4, NI4, 4, w, BIN4, OHW4, body)
            else:
                agg_phase(QS, idx8_d, oh8_d, B8, NI8, 8, w, BIN8, OHW8, body)
            zkey = l + 1 if l < 2 else 3
            if os.environ.get("GCN_NO_CC"):
                nc.sync.dma_start(zfullT[zkey][0], zownT[zkey][:])
            else:
                nc.gpsimd.collective_compute(
                    "AllGather", AG.bypass, replica_groups=[list(range(NC))],
                    ins=[zownT[zkey][:].opt()], outs=[zfullT[zkey][:].opt()])

        pool_ps = psP.tile([4, B], dt.float32, name="pool_ps")

        for l in range(3):
            layer(l)

        # ---- layer 4: per-node agg of z4 (8-chunk streams), then pool ----
        for c in range(NC):
            nc.scalar.dma_start(table[16 * c:16 * c + 4, 0:PS], zfullT[3][c])
        state4 = {}

        def body4(bank, t, c, lhsT, oh_t):
            if t == 0 and c == 0:
                state4["agg"] = psA.tile([4, 512], dt.float32, tag="agg",
                                         name="agg4")
            nc.tensor.matmul(state4["agg"][:, BIN8 * t:BIN8 * t + BIN8],
                             lhsT=lhsT,
                             rhs=oh_t[:, (t * 8 + c) * BIN8:(t * 8 + c + 1) * BIN8],
                             start=(c == 0), stop=(c == 7))
            if t == NI8 // 128 - 1 and c == 7:
                pphase(bank, state4["agg"])

        def pphase(bank, agg_ps):
            aggs = sbB.tile([4, 512], dt.float32, tag="aggs", name="agg4s")
            nc.vector.tensor_copy(aggs[:], agg_ps[:])
            pw = sb.tile([128, 4, B], dt.float16, tag="pw", name="pw")
            nc.sync.dma_start(pw[:], poolw_d[bank])
            for j in range(4):
                trp = psT2.tile([128, 4], dt.float32, tag="trp2", name="trp4")
                nc.tensor.transpose(trp[:], aggs[:, 128 * j:128 * j + 128],
                                    ident[:4, :4])
                trs = sbB.tile([128, 4], dt.float16, tag="trs", name="trs")
                nc.vector.tensor_copy(trs[:], trp[:])
                nc.tensor.matmul(pool_ps[:],
                                 lhsT=trs[:], rhs=pw[:, j, :],
                                 start=False,
                                 stop=(bank == B8 - 1 and j == 3))

        agg_phase(PS, idx8p_d, oh8_d, B8, NI8, 8, 4, BIN8, OHW8, body4)

        pooled = sbB.tile([4, B], dt.float32, name="pooled")
        nc.vector.tensor_copy(pooled[:], pool_ps[:])
        nc.sync.dma_start(pool_in[:], pooled[:])
        if os.environ.get("GCN_NO_CC"):
            nc.sync.dma_start(pool_out[:], pool_in[:])
        else:
            nc.gpsimd.collective_compute(
                "AllReduce", AG.add, replica_groups=[list(range(NC))],
                ins=[pool_in[:].opt()], outs=[pool_out[:].opt()])
        res = sbB.tile([4, B], dt.float32, name="res")
        nc.sync.dma_start(res[:], pool_out[:])
        res2 = sbB.tile([4, B], dt.float32, name="res2")
        nc.vector.tensor_scalar(res2[:], res[:], cvt[:], None, AG.add)
        nc.sync.dma_start(out_d[:], res2[:])

    nc.compile()
    return nc


def build(inputs):
    """Host preprocessing + program build. Returns (nc, in_maps)."""
    x = np.asarray(inputs["x"], F32)
    edge_index = np.asarray(inputs["edge_index"])
    batch = np.asarray(inputs["batch"])
    W = [np.asarray(inputs[f"W{i}"], F32) for i in range(1, 5)]
    b = [np.asarray(inputs[f"b{i}"], F32) for i in range(1, 5)]
    a = [np.asarray(inputs[f"a{i}"], F32) for i in range(1, 4)]
    lw1 = np.asarray(inputs["lw1"], F32)
    lb1 = np.asarray(inputs["lb1"], F32)
    lw2 = np.asarray(inputs["lw2"], F32)
    lb2 = np.asarray(inputs["lb2"], F32)

    (cfg, xtab, idx8, idx8p, oh8, idx4, oh4, poolw, pool2w,
     xself, d2q) = _preprocess(x, edge_index, batch)

    W4p = (W[3] @ lw1 @ lw2).astype(F32)                     # [64, 4]
    cv = (b[3] @ lw1 @ lw2 + lb1 @ lw2 + lb2).astype(F32)    # [4]

    nc = _build_program(cfg)

    in_maps = []
    for c in range(NC):
        m = dict(
            xtab=xtab, idx8=idx8[c], idx8p=idx8p[c], oh8=oh8[c],
            idx4=idx4[c], oh4=oh4[c],
            poolw=poolw[c], pool2w=pool2w[c], xself=xself[c], d2q=d2q[c],
            W1=W[0], W2=W[1], W3=W[2], W4=W4p,
            b1=b[0].reshape(-1, 1), b2=b[1].reshape(-1, 1), b3=b[2].reshape(-1, 1),
            a1=np.full((8, 1), a[0][0], F32),
            a2=np.full((32, 1), a[1][0], F32),
            a3=np.full((64, 1), a[2][0], F32),
            cvec=cv.reshape(4, 1),
        )
        in_maps.append(m)
    return nc, in_maps


def kernel(**inputs):
    nc, in_maps = build(inputs)
    from concourse.bass_utils import run_bass_kernel_spmd
    res = run_bass_kernel_spmd(nc, in_maps, list(range(NC)))
    outT = res.results[0]["out"]      # [4, B]
    return np.ascontiguousarray(outT.T.astype(F32))          # [B, 4]


# revision 67
# speedup vs baseline: 3.5645x; 1.0607x over previous
"""BaseGCN (4-layer GCN + mean-pool + MLP) on 8 Trainium2 NeuronCores.

Strategy: dst-sharded graph parallel, GPSIMD ap_gather message gathering,
PE one-hot scatter matmuls.

  - z tables live in SBUF feature-major: [128 partitions, NEL] where a
    partition-row group holds one src chunk's features.  Layers 1/2/4
    aggregate at width <= 8 and use EIGHT chunks (chunk = src core, 16
    partition rows each) so each gather column serves 8 edges; layer 3
    (width 32) uses FOUR chunks (32 rows each, 2 cores per chunk).
  - dst nodes are packed into "bins" (<= 64 nodes for the 8-chunk group,
    <= 32 for the 4-chunk group) such that every (bin, chunk) has <= 128
    edges; a bin owns one 128-column block of the gather stream, shared
    by all chunks.
  - GPSIMD ap_gather pulls per-edge columns feature-major; PE transposes
    128-col slabs to edge-major; K=128 matmuls against streamed one-hot
    tiles accumulate agg^T [w, 512] per bank in PSUM.
  - Weights/bias/PReLU applied in transposed space; z^T written per-bank
    to DRAM; AllGather exchanges z^T between layers.
  - Layer 4 aggregates z4 = h3 @ (W4 lw1 lw2) at width 4 reusing the
    8-chunk streams, then pools via per-bank transposes + matmuls against
    a static [slot, graph] 1/cnt matrix; AllReduce + constant finishes.

GCNConv(x) = A_hat (x W) + b with A_hat = D^-1/2 A D^-1/2 + D^-1 I;
aggregation commutes with the weight matmul so we aggregate at
width min(d_in, d_out): widths 8, 8, 32, 4.
"""

import os
import numpy as np

# ---------------- problem constants (hardcoded per the contract) ----------
N = 100000
E = 1600000
B = 64
NC = 8
NPC = N // NC          # 12500 dst nodes per core
CAP = 128              # edges per (bin, chunk) == columns per bin
BIN8 = 64              # node columns per bin, 8-chunk group
BIN4 = 32              # node columns per bin, 4-chunk group
NI8 = 1024             # gather columns per bank, 8-chunk group (8 bins)
NI4 = 2048             # gather columns per bank, 4-chunk group (16 bins)
D_IN = 8
DIMS = [8, 32, 64]             # h widths for layers 1..3
AGG_W = [8, 8, 32, 4]          # aggregation widths per layer
F32 = np.float32


def _bin_nodes(sizes, maxn, group):
    """Balanced multiway packing: fix a target bin count, then place each
    node (desc by load) into the feasible bin minimizing the resulting max
    chunk load.  sizes: [n, CH] int.  Returns (bin_of, col_of, n_bins)."""
    n, ch = sizes.shape
    tot = sizes.sum(1)
    target = max(int(np.ceil(n / maxn)),
                 int(np.ceil(sizes.sum(0).max() * 1.035 / CAP)),
                 int(np.ceil(sizes.sum() * 1.03 / (ch * CAP))))
    target = int(np.ceil(target / group) * group)
    order = np.argsort(-tot, kind="stable")
    cap_bins = target + 4 * group
    loads = np.zeros((cap_bins, ch), np.int64)
    cnts = np.zeros(cap_bins, np.int64)
    nbins = target
    bin_of = np.zeros(n, np.int32)
    col_of = np.zeros(n, np.int32)
    for v in order:
        s = sizes[v]
        cand = loads[:nbins] + s
        mx = cand.max(1)
        feas = (cand <= CAP).all(1) & (cnts[:nbins] < maxn)
        if feas.any():
            mx[~feas] = 1 << 30
            bi = int(np.argmin(mx))
        else:
            bi = nbins
            nbins += 1
        bin_of[v] = bi
        col_of[v] = cnts[bi]
        loads[bi] += s
        cnts[bi] += 1
    return bin_of, col_of, nbins


def _preprocess(x, edge_index, batch):
    src = edge_index[0].astype(np.int64)
    dst = edge_index[1].astype(np.int64)
    batch = batch.astype(np.int64)

    deg = np.bincount(dst, minlength=N).astype(F32) + 1.0
    dinv = (1.0 / np.sqrt(deg)).astype(F32)
    dinv2 = (dinv * dinv).astype(F32)

    # edge-only streams for the 8-chunk group (self loops handled via
    # algebraic diagonal terms); self loops stay in-stream for layer 3.
    allsrc = np.concatenate([src, np.arange(N, dtype=np.int64)])
    alldst = np.concatenate([dst, np.arange(N, dtype=np.int64)])
    allval = np.concatenate([dinv[src] * dinv[dst], dinv2]).astype(F32)
    is_edge = np.concatenate([np.ones(len(src), bool), np.zeros(N, bool)])

    cnt = np.maximum(np.bincount(batch, minlength=B).astype(F32), 1.0)

    sc8 = (allsrc // NPC).astype(np.int64)            # src core = 8-chunk id
    sc4 = sc8 // 2                                    # 4-chunk id
    e_core = (alldst // NPC).astype(np.int64)

    # ---- per-core binning (edges only; one bin structure for all layers) --
    bin8_of = np.zeros(N, np.int32); col8_of = np.zeros(N, np.int32)
    nb8 = []
    for c in range(NC):
        lo, hi = c * NPC, (c + 1) * NPC
        me = (alldst >= lo) & (alldst < hi) & is_edge
        d8 = np.zeros((NPC, 8), np.int64)
        np.add.at(d8, (alldst[me] - lo, sc8[me]), 1)
        b_o, c_o, nb = _bin_nodes(d8, BIN8, 8)
        bin8_of[lo:hi] = b_o; col8_of[lo:hi] = c_o; nb8.append(nb)
    NQB = int(np.ceil(max(nb8) / 8) * 8)         # bins count (x8 per bank)
    B8 = NQB // 8
    QS = NQB * BIN8                              # qs slots per core
    assert QS <= 16384, QS

    qs_slot = (bin8_of.astype(np.int64) * BIN8 + col8_of)   # core-local

    OHW8 = 8 * 8 * BIN8      # 4096
    idx8 = np.zeros((NC, B8, 128, NI8 // 16), np.int16)
    oh8 = np.zeros((NC, B8, 128, OHW8), np.float16)

    for c in range(NC):
        m = (e_core == c) & is_edge
        ev = allval[m]
        esrc = allsrc[m]
        edst = alldst[m]
        esc8 = sc8[m]
        key = bin8_of[edst].astype(np.int64) * 8 + esc8
        order = np.argsort(key, kind="stable")
        ks = key[order]
        starts = np.searchsorted(ks, np.arange(NQB * 8))
        pos = np.arange(len(ks)) - starts[ks]
        assert pos.max() < CAP
        bn, ch = ks // 8, ks % 8
        col = bn * CAP + pos
        bank, cb = col // NI8, col % NI8
        idx8[c, bank, 16 * ch + cb % 16, cb // 16] = qs_slot[esrc[order]]
        blk = (cb // 128)
        oh8[c, bank, cb % 128,
            (blk * 8 + ch) * BIN8 + col8_of[edst[order]]] = ev[order]

    # layer-4 pooling one-hots in QS space: [B8 banks, 128, 4, B] fp16
    poolw = np.zeros((NC, B8, 128, 4, B), np.float16)    # 1/cnt
    pool2w = np.zeros((NC, B8, 128, 4, B), np.float16)   # dinv2/cnt (selfs)
    # xself: x * dinv2 in QS columns (layer-1 diagonal term)
    xself = np.zeros((NC, 8, QS), F32)
    # dinv2 in QS columns (layer-2/3 diagonal terms); rows 0:16 and 32:48
    # carry dinv2 for the padded 48-row layer-3 layout, middle rows zero
    d2q = np.zeros((NC, 48, QS), F32)
    # x in 8-chunk table layout [128, QS]
    xtab = np.zeros((128, QS), F32)
    for c in range(NC):
        nodes = np.arange(c * NPC, (c + 1) * NPC)
        s = qs_slot[nodes]
        g = batch[nodes]
        poolw[c, s // 512, s % 128, (s % 512) // 128, g] = \
            (1.0 / cnt[g]).astype(np.float16)
        pool2w[c, s // 512, s % 128, (s % 512) // 128, g] = \
            (dinv2[nodes] / cnt[g]).astype(np.float16)
        xself[c, :, s] = x[nodes] * dinv2[nodes][:, None]
        d2q[c, :16][:, s] = dinv2[nodes][None, :].repeat(16, 0)
        d2q[c, 32:48][:, s] = dinv2[nodes][None, :].repeat(16, 0)
        xtab[16 * c:16 * c + D_IN, s] = x[nodes].T

    cfg = dict(B8=B8, QS=QS)
    return cfg, xtab, idx8, oh8, poolw, pool2w, xself, d2q


def _build_program(cfg):
    import concourse.bacc as bacc
    import concourse.tile as tile
    import concourse.bass as bass
    import concourse.mybir as mybir
    from concourse.masks import make_identity
    from contextlib import ExitStack

    dt = mybir.dt
    B8, QS = cfg["B8"], cfg["QS"]
    OHW8 = 8 * 8 * BIN8

    nc = bacc.Bacc("TRN2", target_bir_lowering=False, debug=False, num_devices=NC)

    xtab_d = nc.dram_tensor("xtab", [128, QS], dt.float32, kind="ExternalInput")
    idx8_d = nc.dram_tensor("idx8", [B8, 128, NI8 // 16], dt.int16, kind="ExternalInput")
    oh8_d = nc.dram_tensor("oh8", [B8, 128, OHW8], dt.float16, kind="ExternalInput")
    poolw_d = nc.dram_tensor("poolw", [B8, 128, 4, B], dt.float16,
                             kind="ExternalInput")
    pool2w_d = nc.dram_tensor("pool2w", [B8, 128, 4, B], dt.float16,
                              kind="ExternalInput")
    xself_d = nc.dram_tensor("xself", [8, QS], dt.float32, kind="ExternalInput")
    d2q_d = nc.dram_tensor("d2q", [48, QS], dt.float32, kind="ExternalInput")
    Wd = {}
    for i, (ki, ko) in enumerate([(8, 8), (8, 32), (48, 64), (64, 4)]):
        Wd[i] = nc.dram_tensor(f"W{i+1}", [ki, ko], dt.float32, kind="ExternalInput")
    bd, ad = {}, {}
    for i, d in enumerate(DIMS):
        bd[i] = nc.dram_tensor(f"b{i+1}", [d, 1], dt.float32, kind="ExternalInput")
        ad[i] = nc.dram_tensor(f"a{i+1}", [d, 1], dt.float32, kind="ExternalInput")
    cvec_d = nc.dram_tensor("cvec", [4, 1], dt.float32, kind="ExternalInput")
    out_d = nc.dram_tensor("out", [4, B], dt.float32, kind="ExternalOutput")

    AG = mybir.AluOpType

    with tile.TileContext(nc) as tc, ExitStack() as ctx:
        wpool = ctx.enter_context(tc.tile_pool(name="weights", bufs=1))
        dram = ctx.enter_context(tc.tile_pool(name="dram", bufs=1, space="DRAM"))
        sb = ctx.enter_context(tc.tile_pool(name="sb", bufs=3))
        sbB = ctx.enter_context(tc.tile_pool(name="sbB", bufs=2))
        psA = ctx.enter_context(tc.tile_pool(name="psA", bufs=2, space="PSUM"))
        psB = ctx.enter_context(tc.tile_pool(name="psB", bufs=1, space="PSUM"))
        psC = ctx.enter_context(tc.tile_pool(name="psC", bufs=1, space="PSUM"))
        psT = ctx.enter_context(tc.tile_pool(name="psT", bufs=2, space="PSUM"))
        psT2 = ctx.enter_context(tc.tile_pool(name="psT2", bufs=1, space="PSUM"))
        psP = ctx.enter_context(tc.tile_pool(name="psP", bufs=1, space="PSUM"))

        table = wpool.tile([128, 2 * QS], dt.float32, name="table")
        ident = wpool.tile([128, 128], dt.float32, name="ident")
        make_identity(nc, ident[:])

        Wt, bt, at = {}, {}, {}
        for i, (ki, ko) in enumerate([(8, 8), (8, 32), (48, 64), (64, 4)]):
            Wt[i] = wpool.tile([ki, ko], dt.float32, tag=f"w{i}", name=f"wt{i}")
            nc.sync.dma_start(Wt[i][:], Wd[i][:])
        for i, d in enumerate(DIMS):
            bt[i] = wpool.tile([d, 1], dt.float32, tag=f"b{i}", name=f"bt{i}")
            nc.sync.dma_start(bt[i][:], bd[i][:])
            at[i] = wpool.tile([d, 1], dt.float32, tag=f"a{i}", name=f"at{i}")
            nc.sync.dma_start(at[i][:], ad[i][:])
        cvt = wpool.tile([4, 1], dt.float32, name="cvt")
        nc.sync.dma_start(cvt[:], cvec_d[:])

        zownT = {1: dram.tile([8, QS], dt.float32, name="zo1"),
                 2: dram.tile([32, QS], dt.float32, name="zo2"),
                 3: dram.tile([4, QS], dt.float32, name="zo3")}
        zfullT = {1: dram.tile([NC, 8, QS], dt.float32, name="zf1"),
                  2: dram.tile([NC, 32, QS], dt.float32, name="zf2"),
                  3: dram.tile([NC, 4, QS], dt.float32, name="zf3")}
        pool_in = dram.tile([4, B], dt.float32, name="pin")
        pool_out = dram.tile([4, B], dt.float32, name="pout")

        def agg_phase(nel, tab_offs, w, nbanks, body):
            """Gather+transpose+scatter for nbanks banks over the 8-chunk
            streams.  One gather per table half in tab_offs; body(bank, h, t,
            c, lhsT_ap, oh_tile) emits one K=128 matmul per (block, chunk)."""
            for s in range(nbanks):
                idx_t = sb.tile([128, NI8 // 16], dt.int16, tag="idx", name="idx")
                nc.sync.dma_start(idx_t[:], idx8_d[s])
                oh_t = sb.tile([128, OHW8], dt.float16, tag="oh", name="oh")
                nc.sync.dma_start(oh_t[:], oh8_d[s])
                for h, off in enumerate(tab_offs):
                    msgT = sb.tile([128, NI8], dt.float32, tag="msg", name="msg")
                    nc.gpsimd.ap_gather(msgT[:], table[:, off:off + nel],
                                        idx_t[:], channels=128, num_elems=nel,
                                        d=1, num_idxs=NI8)
                    for sg in range(2):
                        trp = psT.tile([128, 512], dt.float32, tag="trp",
                                       name="trp")
                        for jp in range(4):
                            nc.tensor.transpose(
                                trp[:, jp * 128:jp * 128 + 128],
                                msgT[:, 128 * (sg * 4 + jp):128 * (sg * 4 + jp) + 128],
                                ident[:])
                        slabs = sbB.tile([128, 512], dt.float16, tag="slabs",
                                         name="slabs")
                        nc.vector.tensor_copy(slabs[:], trp[:])
                        for jp in range(4):
                            t = sg * 4 + jp
                            for c in range(8):
                                body(s, h, t, c,
                                     slabs[:, jp * 128 + 16 * c:jp * 128 + 16 * c + w],
                                     oh_t)

        def layer(l):  # l = 0, 1, 2
            w = AGG_W[l]
            d = DIMS[l]
            if l == 0:
                nc.scalar.dma_start(table[:, 0:QS], xtab_d[:])
            elif l == 1:
                for c in range(NC):
                    nc.scalar.dma_start(table[16 * c:16 * c + 8, 0:QS],
                                        zfullT[1][c])
            else:
                for c in range(NC):
                    nc.scalar.dma_start(table[16 * c:16 * c + 16, 0:QS],
                                        zfullT[2][c, 0:16])
                    nc.scalar.dma_start(table[16 * c:16 * c + 16, QS:2 * QS],
                                        zfullT[2][c, 16:32])

            state = {}

            wh = 16 if l == 2 else w

            def body(bank, h, t, c, lhsT, oh_t):
                if h == 0 and t == 0 and c == 0:
                    state["agg"] = psA.tile([64, 512], dt.float32, tag="agg",
                                            name="agg")
                nc.tensor.matmul(
                    state["agg"][32 * h:32 * h + wh,
                                 BIN8 * t:BIN8 * t + BIN8],
                    lhsT=lhsT,
                    rhs=oh_t[:, (t * 8 + c) * BIN8:(t * 8 + c + 1) * BIN8],
                    start=(c == 0), stop=(c == 7))
                if l == 2:
                    done = (h == 1 and t == 7 and c == 7)
                else:
                    done = (t == 7 and c == 7)
                if done:
                    bphase(bank, state["agg"])

            def bphase(bank, agg_ps):
                aggs = sbB.tile([48 if l == 2 else w, 512], dt.float32,
                                tag="aggs", name="aggs")
                if l == 0:
                    xs = sb.tile([8, 512], dt.float32, tag="xs", name="xs")
                    nc.sync.dma_start(xs[:], xself_d[:, 512 * bank:512 * bank + 512])
                    nc.vector.tensor_add(aggs[:], agg_ps[0:8, :], xs[:])
                elif l == 1:
                    zs = sb.tile([8, 512], dt.float32, tag="xs", name="zs")
                    nc.sync.dma_start(zs[:], zownT[1][:, 512 * bank:512 * bank + 512])
                    d2 = sb.tile([8, 512], dt.float32, tag="d2", name="d2")
                    nc.sync.dma_start(d2[:], d2q_d[0:8, 512 * bank:512 * bank + 512])
                    zsd = sbB.tile([8, 512], dt.float32, tag="zsd", name="zsd")
                    nc.vector.tensor_mul(zsd[:], zs[:], d2[:])
                    nc.vector.tensor_add(aggs[:], agg_ps[0:8, :], zsd[:])
                else:
                    # halves live at partitions 0:16 and 32:48; contract with
                    # a host-padded [48, 64] W3 (rows 16:32 zero)
                    zs = sb.tile([48, 512], dt.float32, tag="xs3", name="zs3")
                    nc.sync.dma_start(zs[0:16, :],
                                      zownT[2][0:16, 512 * bank:512 * bank + 512])
                    nc.sync.dma_start(zs[32:48, :],
                                      zownT[2][16:32, 512 * bank:512 * bank + 512])
                    d2 = sb.tile([48, 512], dt.float32, tag="d23", name="d23")
                    nc.sync.dma_start(d2[:], d2q_d[:, 512 * bank:512 * bank + 512])
                    zsd = sbB.tile([48, 512], dt.float32, tag="zsd3", name="zsd3")
                    nc.vector.tensor_mul(zsd[:], zs[:], d2[:])
                    nc.vector.memset(aggs[:], 0.0)
                    nc.vector.tensor_add(aggs[0:16, :], agg_ps[0:16, :],
                                         zsd[0:16, :])
                    nc.vector.tensor_add(aggs[32:48, :], agg_ps[32:48, :],
                                         zsd[32:48, :])
                h_ps = psB.tile([d, 512], dt.float32, tag="h", name="h")
                nc.tensor.matmul(h_ps[:], lhsT=Wt[l][:], rhs=aggs[:],
                                 start=True, stop=True)
                neg = sbB.tile([d, 512], dt.float32, tag="neg", name="neg")
                nc.vector.tensor_scalar(neg[:], h_ps[:], bt[l][:], 0.0, AG.add, AG.min)
                nega = sbB.tile([d, 512], dt.float32, tag="nega", name="nega")
                nc.vector.tensor_scalar(nega[:], neg[:], at[l][:], None, AG.mult)
                pos = sbB.tile([d, 512], dt.float32, tag="pos", name="pos")
                nc.vector.tensor_scalar(pos[:], h_ps[:], bt[l][:], 0.0, AG.add, AG.max)
                hT = sbB.tile([d, 512], dt.float32, tag="hT", name="hT")
                nc.vector.tensor_add(hT[:], pos[:], nega[:])
                if l == 2:
                    z4_ps = psC.tile([4, 512], dt.float32, tag="z4", name="z4")
                    nc.tensor.matmul(z4_ps[:], lhsT=Wt[3][:], rhs=hT[:],
                                     start=True, stop=True)
                    z4s = sbB.tile([4, 512], dt.float32, tag="z4s", name="z4s")
                    nc.vector.tensor_copy(z4s[:], z4_ps[:])
                    nc.scalar.dma_start(zownT[3][:, 512 * bank:512 * bank + 512],
                                        z4s[:])
                    # layer-4 self-loop term: pool dinv2*z4 directly
                    p2 = sb.tile([128, 4, B], dt.float16, tag="pw", name="p2w")
                    nc.sync.dma_start(p2[:], pool2w_d[bank])
                    for j in range(4):
                        trp2 = psT2.tile([128, 4], dt.float32, tag="trp2",
                                         name="trp2")
                        nc.tensor.transpose(trp2[:],
                                            z4s[:, 128 * j:128 * j + 128],
                                            ident[:4, :4])
                        trs2 = sbB.tile([128, 4], dt.float16, tag="trs",
                                        name="trs2")
                        nc.vector.tensor_copy(trs2[:], trp2[:])
                        nc.tensor.matmul(pool_ps[:],
                                         lhsT=trs2[:], rhs=p2[:, j, :],
                                         start=(bank == 0 and j == 0),
                                         stop=False)
                else:
                    nc.scalar.dma_start(
                        zownT[l + 1][:, 512 * bank:512 * bank + 512], hT[:])

            if l == 2:
                agg_phase(QS, [0, QS], 16, B8, body)
            else:
                agg_phase(QS, [0], w, B8, body)
            zkey = l + 1 if l < 2 else 3
            if os.environ.get("GCN_NO_CC"):
                nc.sync.dma_start(zfullT[zkey][0], zownT[zkey][:])
            else:
                nc.gpsimd.collective_compute(
                    "AllGather", AG.bypass, replica_groups=[list(range(NC))],
                    ins=[zownT[zkey][:].opt()], outs=[zfullT[zkey][:].opt()])

        pool_ps = psP.tile([4, B], dt.float32, name="pool_ps")

        for l in range(3):
            layer(l)

        # ---- layer 4: per-node agg of z4 (8-chunk streams), then pool ----
        for c in range(NC):
            nc.scalar.dma_start(table[16 * c:16 * c + 4, 0:QS], zfullT[3][c])
        state4 = {}

        def body4(bank, h, t, c, lhsT, oh_t):
            if t == 0 and c == 0:
                state4["agg"] = psA.tile([64, 512], dt.float32, tag="agg",
                                         name="agg4")
            nc.tensor.matmul(state4["agg"][0:4, BIN8 * t:BIN8 * t + BIN8],
                             lhsT=lhsT,
                             rhs=oh_t[:, (t * 8 + c) * BIN8:(t * 8 + c + 1) * BIN8],
                             start=(c == 0), stop=(c == 7))
            if t == NI8 // 128 - 1 and c == 7:
                pphase(bank, state4["agg"])

        def pphase(bank, agg_ps):
            aggs = sbB.tile([4, 512], dt.float32, tag="aggs", name="agg4s")
            nc.vector.tensor_copy(aggs[:], agg_ps[0:4, :])
            pw = sb.tile([128, 4, B], dt.float16, tag="pw", name="pw")
            nc.sync.dma_start(pw[:], poolw_d[bank])
            for j in range(4):
                trp = psT2.tile([128, 4], dt.float32, tag="trp2", name="trp4")
                nc.tensor.transpose(trp[:], aggs[:, 128 * j:128 * j + 128],
                                    ident[:4, :4])
                trs = sbB.tile([128, 4], dt.float16, tag="trs", name="trs")
                nc.vector.tensor_copy(trs[:], trp[:])
                nc.tensor.matmul(pool_ps[:],
                                 lhsT=trs[:], rhs=pw[:, j, :],
                                 start=False,
                                 stop=(bank == B8 - 1 and j == 3))

        agg_phase(QS, [0], 4, B8, body4)

        pooled = sbB.tile([4, B], dt.float32, name="pooled")
        nc.vector.tensor_copy(pooled[:], pool_ps[:])
        nc.sync.dma_start(pool_in[:], pooled[:])
        if os.environ.get("GCN_NO_CC"):
            nc.sync.dma_start(pool_out[:], pool_in[:])
        else:
            nc.gpsimd.collective_compute(
                "AllReduce", AG.add, replica_groups=[list(range(NC))],
                ins=[pool_in[:].opt()], outs=[pool_out[:].opt()])
        res = sbB.tile([4, B], dt.float32, name="res")
        nc.sync.dma_start(res[:], pool_out[:])
        res2 = sbB.tile([4, B], dt.float32, name="res2")
        nc.vector.tensor_scalar(res2[:], res[:], cvt[:], None, AG.add)
        nc.sync.dma_start(out_d[:], res2[:])

    nc.compile()
    return nc


def build(inputs):
    """Host preprocessing + program build. Returns (nc, in_maps)."""
    x = np.asarray(inputs["x"], F32)
    edge_index = np.asarray(inputs["edge_index"])
    batch = np.asarray(inputs["batch"])
    W = [np.asarray(inputs[f"W{i}"], F32) for i in range(1, 5)]
    b = [np.asarray(inputs[f"b{i}"], F32) for i in range(1, 5)]
    a = [np.asarray(inputs[f"a{i}"], F32) for i in range(1, 4)]
    lw1 = np.asarray(inputs["lw1"], F32)
    lb1 = np.asarray(inputs["lb1"], F32)
    lw2 = np.asarray(inputs["lw2"], F32)
    lb2 = np.asarray(inputs["lb2"], F32)

    (cfg, xtab, idx8, oh8, poolw, pool2w,
     xself, d2q) = _preprocess(x, edge_index, batch)

    W4p = (W[3] @ lw1 @ lw2).astype(F32)                     # [64, 4]
    W3p = np.zeros((48, 64), F32)                            # padded rows
    W3p[0:16] = W[2][0:16]
    W3p[32:48] = W[2][16:32]
    cv = (b[3] @ lw1 @ lw2 + lb1 @ lw2 + lb2).astype(F32)    # [4]

    nc = _build_program(cfg)

    in_maps = []
    for c in range(NC):
        m = dict(
            xtab=xtab, idx8=idx8[c], oh8=oh8[c],
            poolw=poolw[c], pool2w=pool2w[c], xself=xself[c], d2q=d2q[c],
            W1=W[0], W2=W[1], W3=W3p, W4=W4p,
            b1=b[0].reshape(-1, 1), b2=b[1].reshape(-1, 1), b3=b[2].reshape(-1, 1),
            a1=np.full((8, 1), a[0][0], F32),
            a2=np.full((32, 1), a[1][0], F32),
            a3=np.full((64, 1), a[2][0], F32),
            cvec=cv.reshape(4, 1),
        )
        in_maps.append(m)
    return nc, in_maps


def kernel(**inputs):
    nc, in_maps = build(inputs)
    from concourse.bass_utils import run_bass_kernel_spmd
    res = run_bass_kernel_spmd(nc, in_maps, list(range(NC)))
    outT = res.results[0]["out"]      # [4, B]
    return np.ascontiguousarray(outT.T.astype(F32))          # [B, 4]
